# revision 1
# baseline (speedup 1.0000x reference)
"""CRF-RNN layer (nn_CrfRnnLayer) as a Bass/Tile SPMD kernel on 8 TRN2 NeuronCores.

Algorithm (matches reference.py):
  N = 112*112 pixels, C = 21 classes, 5 mean-field iterations:
    sm = softmax(Q, axis=classes)
    spatial_out  = (sm @ Ks) / ns      Ks[i,j] = exp(-||p_i-p_j||^2 / (2*3^2))
    bilateral_out= (sm @ Kb) / nb      Kb from (pos/160, rgb/3) features
    Q = u - comp @ (sk @ spatial_out + bk @ bilateral_out)

Sharding: pixel columns sharded 8 ways (each core owns 14 image rows = 1568
pixels). Each core computes its column slice of both filtered outputs from the
fully replicated softmax; a per-iteration AllGather (131KB/rank) replicates Q.

Structure per core (instruction-count optimized):
  - Bilateral slice E_b [N, 1568] precomputed once on-device in bf16 (K=7
    matmul of augmented features -> -0.5*d2 into a 4-bank PSUM tile, one ACT
    exp over all 4 banks) and streamed back each iteration in 7-block batches.
  - Main bilateral matmul: 98 contraction blocks x 4 col-tiles into a single
    4-bank [33, 2048] PSUM accumulator; softmax lhsT carries a ones column at
    partition 32 whose PSUM row is the nb normalizer (read once, iteration 0).
  - Spatial kernel never materialized: Ks = Gy (x) Gx Kronecker product. One
    big DVE multiply (per half) scales the softmax image by the per-core
    y-Gaussian weights; 112 PE matmuls against the shared [112,112] x-Gaussian
    do the rest. Normalizers ns are exact separable sums computed on host.
  - Layout changes (pixel-major <-> block-major <-> class-major) via single
    strided DMAs through DRAM instead of per-tile PE transposes.
"""

import numpy as np

import concourse.mybir as mybir
import concourse.tile as tile
from concourse import bacc
from concourse.bass import _add_dep_helper
from concourse.bass_utils import run_bass_kernel_spmd

H = 112
W = 112
C = 21
N = H * W
NCORES = 8
YPC = H // NCORES            # 14 image rows per core
COLS = N // NCORES           # 1568 pixels per core
NB = 98                      # contraction blocks of 128 pixels
BB = 7                       # blocks per DMA batch
NBATCH = NB // BB            # 14
CTS = [(0, 512), (512, 512), (1024, 512), (1536, 32)]
NITER = 5
THETA_ALPHA = 160.0
THETA_BETA = 3.0
THETA_GAMMA = 3.0
CP = 33                      # padded lhsT width; col 32 is the ones column

F32 = mybir.dt.float32
BF16 = mybir.dt.bfloat16
EXPF = mybir.ActivationFunctionType.Exp

_CACHE = {}


def _build_program(reps=1):
    nc = bacc.Bacc("TRN2", target_bir_lowering=False, debug=False, num_devices=NCORES)

    # Chain every PE matmul in emission order (ordering-only deps) so the
    # scheduler keeps same-weights matmuls adjacent -> legalization dedups
    # the LDWEIGHTS instruction for consecutive same-lhsT matmuls.
    _mm_state = {"prev": None}

    def mm(*args, **kwargs):
        inst = nc.tensor.matmul(*args, **kwargs)
        if _mm_state["prev"] is not None:
            _add_dep_helper(inst.ins, _mm_state["prev"].ins, sync=False,
                            reason="pe emission order")
        _mm_state["prev"] = inst
        return inst

    ubT = nc.dram_tensor("ubT", [7, N], F32, kind="ExternalInput")
    vbT_sl = nc.dram_tensor("vbT_sl", [7, COLS], F32, kind="ExternalInput")
    g2d = nc.dram_tensor("g2d", [H, W], F32, kind="ExternalInput")
    gy2 = nc.dram_tensor("gy2", [H, YPC], F32, kind="ExternalInput")
    invns2 = nc.dram_tensor("invns2", [YPC, W * C], F32, kind="ExternalInput")
    u_sl = nc.dram_tensor("u_sl", [C, COLS], F32, kind="ExternalInput")
    qt0 = nc.dram_tensor("qt0", [W, H * C], F32, kind="ExternalInput")
    awT = nc.dram_tensor("awT", [54, C], F32, kind="ExternalInput")
    qt_out = nc.dram_tensor("qt_out", [C, COLS], F32, kind="ExternalOutput")

    with tile.TileContext(nc) as tc:
        with (
            tc.tile_pool(name="const", bufs=1) as cpool,
            tc.tile_pool(name="smx", bufs=1) as smpool,
            tc.tile_pool(name="stream", bufs=2) as stpool,
            tc.tile_pool(name="outp", bufs=1) as opool,
            tc.tile_pool(name="psum", bufs=1, space="PSUM") as pspool,
            tc.tile_pool(name="dram", bufs=1, space="DRAM") as dpool,
        ):
          for _rep in range(reps):
            # ---------------- constants ----------------
            vbT_sb = cpool.tile([7, COLS], F32, tag="vbT", name=f"vbT_{_rep}")
            nc.sync.dma_start(vbT_sb[:], vbT_sl[:])
            u_sb = cpool.tile([C, COLS], F32, tag="usb", name=f"usb_{_rep}")
            nc.sync.dma_start(u_sb[:], u_sl[:])
            awT_sb = cpool.tile([54, C], F32, tag="awT", name=f"awT_{_rep}")
            nc.sync.dma_start(awT_sb[:], awT[:])
            invns2_sb = cpool.tile([YPC, W * C], F32, tag="invns2", name=f"invns2_{_rep}")
            nc.sync.dma_start(invns2_sb[:], invns2[:])
            gy2_sb = cpool.tile([H, YPC], F32, tag="gy2", name=f"gy2_{_rep}")
            nc.sync.dma_start(gy2_sb[:], gy2[:])

            g2d_f = cpool.tile([H, W], F32, tag="g2df", name=f"g2df_{_rep}")
            nc.sync.dma_start(g2d_f[:], g2d[:])


            ones1 = cpool.tile([1, C], F32, tag="ones1", name=f"ones1_{_rep}")
            nc.gpsimd.memset(ones1[:], 1.0)
            invnb_bc = cpool.tile([C, COLS], F32, tag="invnb", name=f"invnb_{_rep}")

            # DRAM scratch
            e_b = dpool.tile([NB, 128, COLS], F32, tag="eb", name=f"eb_{_rep}")
            sm_d = dpool.tile([N, C], F32, tag="smd", name=f"smd_{_rep}")
            td_d = dpool.tile([H, W * C], F32, tag="td", name=f"td_{_rep}")
            spd = dpool.tile([W * C, YPC], F32, tag="spd", name=f"spd_{_rep}")

            # ---------------- precompute E_b (98 blocks, batches of 7) ------
            for bt in range(NBATCH):
                ub7 = stpool.tile([7, BB * 128], F32, tag="ub7", name=f"ub7_{_rep}_{bt}")
                nc.sync.dma_start(
                    ub7[:], ubT[:, bt * BB * 128 : (bt + 1) * BB * 128]
                )
                e7 = stpool.tile(
                    [128, BB, COLS], F32, tag="e7", bufs=1, name=f"e7p_{_rep}_{bt}"
                )
                for b in range(BB):
                    tagp = "blk" if b % 2 == 0 else "spq"
                    d2_ps = pspool.tile(
                        [128, 2048], F32, tag=tagp, name=f"d2_{_rep}_{bt}_{b}"
                    )
                    for ci, (c0, cw) in enumerate(CTS):
                        mm(
                            d2_ps[:, ci * 512 : ci * 512 + cw],
                            ub7[:, b * 128 : (b + 1) * 128],
                            vbT_sb[:, c0 : c0 + cw],
                            start=True,
                            stop=True,
                        )
                    nc.scalar.activation(e7[:, b, :], d2_ps[:, 0:COLS], EXPF)
                nc.sync.dma_start(
                    e_b[bt * BB : (bt + 1) * BB].rearrange("b r f -> r b f"), e7[:]
                )

            # ---------------- iterations ----------------
            qt_full = None
            for it in range(NITER):
                # ---- Q in [x, (y c)] layout
                q3f = smpool.tile([W, H * C], F32, tag="q3f", name=f"q3f_{_rep}_{it}")
                if it == 0:
                    nc.sync.dma_start(q3f[:], qt0[:])
                else:
                    nc.sync.dma_start(
                        q3f[:], qt_full[:].rearrange("r x -> x r")
                    )

                # ---- softmax over classes (no max subtraction; |Q| small)
                eq3b = smpool.tile([W, H * C], F32, tag="eq3b", name=f"eq_{_rep}_{it}")
                nc.scalar.activation(eq3b[:], q3f[:], EXPF)
                sums = smpool.tile([W, H], F32, tag="sums", name=f"sums_{_rep}_{it}")
                nc.vector.reduce_sum(
                    sums[:],
                    eq3b[:].rearrange("p (y c) -> p y c", y=H),
                    axis=mybir.AxisListType.X,
                )
                rsum = smpool.tile([W, H], F32, tag="rsum", name=f"rsum_{_rep}_{it}")
                nc.vector.reciprocal(rsum[:], sums[:])
                smT3 = smpool.tile([W, H, C], F32, tag="smT3", name=f"smT3_{_rep}_{it}")
                nc.vector.tensor_mul(
                    smT3[:],
                    eq3b[:].rearrange("p (y c) -> p y c", y=H),
                    rsum[:].broadcast_to([W, H, C]),
                )

                # ---- block-major softmax copy for the bilateral lhsT
                nc.sync.dma_start(
                    sm_d[:].rearrange("(y x) c -> x y c", x=W), smT3[:]
                )
                smB = smpool.tile([128, NB, CP], F32, tag="smB", name=f"smB_{_rep}_{it}")
                nc.gpsimd.memset(smB[:, :, C:CP], 1.0)
                nc.sync.dma_start(
                    smB[:, :, 0:C], sm_d[:].rearrange("(b r) c -> r b c", r=128)
                )

                # ---- spatial filtering: separable x-pass then y-pass
                # pass 1 (x-conv): T[x', (y c)] = Gx^T @ smT3
                p1a = pspool.tile([W, 2048], F32, tag="spq", name=f"p1a_{_rep}_{it}")
                smflat = smT3[:].rearrange("p y c -> p (y c)")
                for ci in range(4):
                    mm(p1a[:, ci * 512 : (ci + 1) * 512], g2d_f[:],
                       smflat[:, ci * 512 : (ci + 1) * 512], start=True, stop=True)
                p1b = pspool.tile([W, 304], F32, tag="spq", name=f"p1b_{_rep}_{it}")
                mm(p1b[:], g2d_f[:], smflat[:, 2048:2352], start=True, stop=True)
                t_sb = smpool.tile([W, H * C], F32, tag="tsb", name=f"tsb_{_rep}_{it}")
                nc.vector.tensor_copy(t_sb[:, 0:2048], p1a[:])
                nc.vector.tensor_copy(t_sb[:, 2048:2352], p1b[:])
                # bounce [x', (y c)] -> [y, (x' c)]
                nc.sync.dma_start(
                    td_d[:].rearrange("y (x c) -> x y c", x=W),
                    t_sb[:].rearrange("p (y c) -> p y c", y=H),
                )
                t2_sb = smpool.tile([H, W * C], F32, tag="t2sb", name=f"t2sb_{_rep}_{it}")
                nc.sync.dma_start(t2_sb[:], td_d[:])
                # pass 2 (y-conv): sp2[k, (x' c)] = gy2^T @ T2, then / ns
                p2a = pspool.tile([YPC, 2048], F32, tag="spq", name=f"p2a_{_rep}_{it}")
                for ci in range(4):
                    mm(p2a[:, ci * 512 : (ci + 1) * 512], gy2_sb[:],
                       t2_sb[:, ci * 512 : (ci + 1) * 512], start=True, stop=True)
                p2b = pspool.tile([YPC, 304], F32, tag="spq", name=f"p2b_{_rep}_{it}")
                mm(p2b[:], gy2_sb[:], t2_sb[:, 2048:2352], start=True, stop=True)
                sp2 = opool.tile([YPC, W * C], F32, tag="sp2", name=f"sp2_{_rep}_{it}")
                nc.vector.tensor_mul(sp2[:, 0:2048], p2a[:], invns2_sb[:, 0:2048])
                nc.vector.tensor_mul(sp2[:, 2048:2352], p2b[:], invns2_sb[:, 2048:2352])

                # ---- bilateral: stream E_b and accumulate [CP, 2048] PSUM
                bl_ps = pspool.tile([CP, 2048], F32, tag="blk", name=f"bl_{_rep}_{it}")
                for bt in range(NBATCH):
                    e7s = stpool.tile(
                        [128, BB, COLS], F32, tag="e7", bufs=1, name=f"e7s_{_rep}_{it}_{bt}"
                    )
                    nc.sync.dma_start(
                        e7s[:], e_b[bt * BB : (bt + 1) * BB].rearrange("b r f -> r b f")
                    )
                    for b in range(BB):
                        jb = bt * BB + b
                        for ci, (c0, cw) in enumerate(CTS):
                            mm(
                                bl_ps[:, ci * 512 : ci * 512 + cw],
                                smB[:, jb, :],
                                e7s[:, b, c0 : c0 + cw],
                                start=(jb == 0),
                                stop=(jb == NB - 1),
                            )

                # ---- iteration 0: build 1/nb broadcast across class partitions
                if it == 0:
                    nbrow = opool.tile([1, COLS], F32, tag="nbrow", name=f"nbrow_{_rep}")
                    nc.vector.tensor_copy(nbrow[:], bl_ps[32:33, 0:COLS])
                    rnb = opool.tile([1, COLS], F32, tag="rnb", name=f"rnb_{_rep}")
                    nc.vector.reciprocal(rnb[:], nbrow[:])
                    bc_ps = pspool.tile([C, 2048], F32, tag="spq", name=f"bc_{_rep}")
                    for ci, (c0, cw) in enumerate(CTS):
                        mm(
                            bc_ps[:, ci * 512 : ci * 512 + cw],
                            ones1[:],
                            rnb[0:1, c0 : c0 + cw],
                            start=True,
                            stop=True,
                        )
                    nc.vector.tensor_copy(invnb_bc[:], bc_ps[:, 0:COLS])

                # ---- stacked [54, COLS]: spatial_out rows 0:21, bilateral 32:53
                so54 = opool.tile([54, COLS], F32, tag="so54", name=f"so54_{_rep}_{it}")
                nc.gpsimd.memset(so54[:], 0.0)
                # 2D-transpose write: spd[(x c), k] <- sp2[k, (x c)]
                nc.sync.dma_start(spd[:].rearrange("r k -> k r"), sp2[:])
                # 3D read: X1[c, x, k] <- spd[(x c), k]
                x1 = opool.tile([C, W, YPC], F32, tag="x1", name=f"x1_{_rep}_{it}")
                nc.sync.dma_start(
                    x1[:], spd[:].rearrange("(x c) k -> c x k", x=W)
                )
                # DVE free-dim transpose (x,k) -> (k,x) into the stacked tile
                nc.vector.tensor_copy(
                    so54[0:C, :].rearrange("c (k x) -> c k x", k=YPC),
                    x1[:].rearrange("c x k -> c k x"),
                )
                # normalized bilateral into rows 32:53
                nc.vector.tensor_mul(so54[32:53, :], bl_ps[0:C, 0:COLS], invnb_bc[:])

                # ---- Q = u + [A_s ; A_b] @ [sp_out ; bl_out]
                q_ps = pspool.tile([C, 2048], F32, tag="spq", name=f"qps_{_rep}_{it}")
                for ci, (c0, cw) in enumerate(CTS):
                    mm(
                        q_ps[:, ci * 512 : ci * 512 + cw],
                        awT_sb[:],
                        so54[:, c0 : c0 + cw],
                        start=True,
                        stop=True,
                    )
                q_sb = opool.tile([C, COLS], F32, tag="qsb", name=f"qsb_{_rep}_{it}")
                nc.vector.tensor_add(q_sb[:], q_ps[:, 0:COLS], u_sb[:])

                # ---- publish Q: AllGather (iters 0-3) or final output
                if it < NITER - 1:
                    qt_sl = dpool.tile(
                        [YPC * C, W], F32, tag="qtsl", bufs=2, name=f"qtsl_{_rep}_{it}"
                    )
                    nc.sync.dma_start(
                        qt_sl[:].rearrange("(k c) x -> c k x", k=YPC),
                        q_sb[:].rearrange("c (k x) -> c k x", k=YPC),
                    )
                    qt_full = dpool.tile(
                        [H * C, W], F32, tag="qtfull", bufs=2,
                        addr_space="Shared", name=f"qtfull_{_rep}_{it}",
                    )
                    nc.gpsimd.collective_compute(
                        "AllGather",
                        mybir.AluOpType.bypass,
                        replica_groups=[list(range(NCORES))],
                        ins=[qt_sl[:]],
                        outs=[qt_full[:]],
                    )
                else:
                    nc.sync.dma_start(qt_out[:], q_sb[:])

    nc.compile()
    return nc


def _host_inputs(unaries, rgb, spatial_kernel, bilateral_kernel, compatibility_matrix):
    u = np.transpose(np.asarray(unaries, dtype=np.float32)[0], (2, 0, 1)).reshape(C, N)
    rgbf = np.asarray(rgb, dtype=np.float32)[0].reshape(N, 3)

    yy, xx = np.meshgrid(
        np.arange(H, dtype=np.float64), np.arange(W, dtype=np.float64), indexing="ij"
    )
    pos = np.stack([xx.ravel(), yy.ravel()], axis=1)  # [N, 2] (x, y)

    fb = np.concatenate(
        [pos / THETA_ALPHA, rgbf.astype(np.float64) / THETA_BETA], axis=1
    )
    fb -= fb.mean(axis=0, keepdims=True)  # centering: reduces fp32 cancellation
    fb32 = fb.astype(np.float32)
    sq = (fb32.astype(np.float64) ** 2).sum(axis=1)
    mhalf_sq = (-0.5 * sq).astype(np.float32)

    ubT = np.empty((7, N), np.float32)
    ubT[0:5] = fb32.T
    ubT[5] = mhalf_sq
    ubT[6] = 1.0
    vbT = np.empty((7, N), np.float32)
    vbT[0:5] = fb32.T
    vbT[5] = 1.0
    vbT[6] = mhalf_sq

    d = np.arange(-(H - 1), H, dtype=np.float64)
    g1tab = np.exp(-(d * d) / (2.0 * THETA_GAMMA**2))

    def g1(dd):
        return g1tab[np.asarray(dd) + (H - 1)]

    gx = g1(np.arange(W)[:, None] - np.arange(W)[None, :])  # [x, x']
    g2d_np = gx.astype(np.float32)
    s1 = np.array([g1(np.arange(H) - t).sum() for t in range(H)])  # exact ns factors

    comp = np.asarray(compatibility_matrix, dtype=np.float64)
    A_s = -(comp @ np.asarray(spatial_kernel, dtype=np.float64))
    A_b = -(comp @ np.asarray(bilateral_kernel, dtype=np.float64))
    awT_np = np.zeros((54, C), np.float32)
    awT_np[0:21] = A_s.T.astype(np.float32)
    awT_np[32:53] = A_b.T.astype(np.float32)

    qt0_np = np.ascontiguousarray(
        u.reshape(C, H, W).transpose(2, 1, 0).reshape(W, H * C)
    )

    in_maps = []
    for c in range(NCORES):
        sl = slice(c * COLS, (c + 1) * COLS)
        dy = np.arange(H)[:, None] - (YPC * c + np.arange(YPC))[None, :]  # [y, k]
        gy2_np = np.ascontiguousarray(g1(dy).astype(np.float32))  # [112, 14]
        # invns2[k, x*21 + cc] = 1 / (s1[y0+k] * s1[x])
        v = 1.0 / (s1[YPC * c + np.arange(YPC)][:, None] * s1[np.arange(W)][None, :])
        invns2_np = np.ascontiguousarray(
            np.repeat(v[:, :, None], C, axis=2).astype(np.float32)
        ).reshape(YPC, W * C)
        in_maps.append(
            dict(
                ubT=ubT,
                vbT_sl=np.ascontiguousarray(vbT[:, sl]),
                g2d=g2d_np,
                gy2=gy2_np,
                invns2=invns2_np,
                u_sl=np.ascontiguousarray(u[:, sl]),
                qt0=qt0_np,
                awT=awT_np,
            )
        )
    return in_maps


def run(inputs, trace=False, reps=1, **spmd_kwargs):
    in_maps = _host_inputs(**inputs)
    key = ("nc", reps)
    if key not in _CACHE:
        _CACHE[key] = _build_program(reps)
    nc = _CACHE[key]
    res = run_bass_kernel_spmd(
        nc, in_maps, core_ids=list(range(NCORES)), trace=trace, **spmd_kwargs
    )
    qs = [np.asarray(res.results[c]["qt_out"]) for c in range(NCORES)]
    Q = np.concatenate(qs, axis=1)  # [C, N]
    out = Q.reshape(C, H, W).transpose(1, 2, 0)[None].astype(np.float32)
    return out, res


def kernel(unaries, rgb, spatial_kernel, bilateral_kernel, compatibility_matrix):
    out, _ = run(
        dict(
            unaries=unaries,
            rgb=rgb,
            spatial_kernel=spatial_kernel,
            bilateral_kernel=bilateral_kernel,
            compatibility_matrix=compatibility_matrix,
        )
    )
    return out



# revision 11
# speedup vs baseline: 2.7367x; 2.7367x over previous
"""CRF-RNN layer (nn_CrfRnnLayer) as a Bass/Tile SPMD kernel on 8 TRN2 NeuronCores.

Algorithm (matches reference.py):
  N = 112*112 pixels, C = 21 classes, 5 mean-field iterations:
    sm = softmax(Q, axis=classes)
    spatial_out  = (sm @ Ks) / ns      Ks[i,j] = exp(-||p_i-p_j||^2 / (2*3^2))
    bilateral_out= (sm @ Kb) / nb      Kb from (pos/160, rgb/3) features
    Q = u - comp @ (sk @ spatial_out + bk @ bilateral_out)

Design (v2):
  - Pixel columns sharded 8 ways (1568 cols/core). Bilateral kernel slice
    E_b = Kb[:, cols] is computed once on-device (fp32r d2 matmul -> ACT exp)
    and stored *fully resident in SBUF as fp8-e4m3* (98 blocks of 128 pixels,
    ~150 KB/partition) -> zero DMA traffic in the main loop.
  - Main bilateral matmul: bf16 softmax lhsT (with a ones column at row 21
    whose PSUM row yields the normalizer nb) x fp8 E blocks = 1 cycle/row.
  - Q is AllGathered in bf16 pixel-major [N, C] layout, so the block-major
    softmax input reloads with a single strided DMA (no transposes).
  - Spatial filtering is separable with the 1/ns normalizers folded into the
    bf16 Gaussian matrices host-side. The softmaxed smB bounces through DRAM
    into [y,(x c)] layout; y-pass matmul, [k,(x c)]->[x,(k c)] bounce, x-pass
    matmul, then a small relayout into the stacked Q-update rhs. The whole
    chain overlaps the bilateral matmul.
  - SBUF pressure handled by tag-sharing temporally disjoint tiles.
"""

import numpy as np
import ml_dtypes

import concourse.mybir as mybir
import concourse.tile as tile
from concourse import bacc
from concourse.bass import _add_dep_helper
from concourse.bass_utils import run_bass_kernel_spmd

H = 112
W = 112
C = 21
N = H * W
NCORES = 8
YPC = H // NCORES            # 14 image rows per core
COLS = N // NCORES           # 1568 pixels per core
NB = 98                      # contraction blocks of 128 pixels
CTS = [(0, 512), (512, 512), (1024, 512), (1536, 32)]   # col tiles of 1568
NITER = 5
THETA_ALPHA = 160.0
THETA_BETA = 3.0
THETA_GAMMA = 3.0

F32 = mybir.dt.float32
F32R = mybir.dt.float32r
BF16 = mybir.dt.bfloat16
FP8 = mybir.dt.float8e4
EXPF = mybir.ActivationFunctionType.Exp

_CACHE = {}


def _build_program():
    nc = bacc.Bacc("TRN2", target_bir_lowering=False, debug=False, num_devices=NCORES)

    # Chain every PE matmul in emission order (ordering-only deps) so the
    # scheduler keeps same-weights matmuls adjacent for LDWEIGHTS dedup.
    _mm_state = {"prev": None}

    def mm(*args, **kwargs):
        inst = nc.tensor.matmul(*args, **kwargs)
        if _mm_state["prev"] is not None:
            _add_dep_helper(inst.ins, _mm_state["prev"].ins, sync=False,
                            reason="pe emission order")
        _mm_state["prev"] = inst
        return inst

    ub7 = nc.dram_tensor("ub7", [7, N], F32R, kind="ExternalInput")
    vb7 = nc.dram_tensor("vb7", [7, COLS], F32R, kind="ExternalInput")
    gxn = nc.dram_tensor("gxn", [W, W], BF16, kind="ExternalInput")
    gy2n = nc.dram_tensor("gy2n", [H, YPC], BF16, kind="ExternalInput")
    u21 = nc.dram_tensor("u21", [C, COLS], F32, kind="ExternalInput")
    qblk0 = nc.dram_tensor("qblk0", [128, NB * C], BF16, kind="ExternalInput")
    qx0 = nc.dram_tensor("qx0", [W, H * C], BF16, kind="ExternalInput")
    awT = nc.dram_tensor("awT", [54, C], F32, kind="ExternalInput")
    qt_out = nc.dram_tensor("qt_out", [C, COLS], F32, kind="ExternalOutput")

    with tile.TileContext(nc) as tc:
        with (
            tc.tile_pool(name="const", bufs=1) as cpool,
            tc.tile_pool(name="iter", bufs=1) as ipool,
            tc.tile_pool(name="stream", bufs=1) as stpool,
            tc.tile_pool(name="psum", bufs=1, space="PSUM") as pspool,
            tc.tile_pool(name="dram", bufs=1, space="DRAM") as dpool,
        ):
            # ---------------- constants ----------------
            gxn_sb = cpool.tile([W, W], BF16, tag="gxn", name="gxn")
            nc.sync.dma_start(gxn_sb[:], gxn[:])
            gy2n_sb = cpool.tile([H, YPC], BF16, tag="gy2n", name="gy2n")
            nc.sync.dma_start(gy2n_sb[:], gy2n[:])
            u21_sb = cpool.tile([C, COLS], F32, tag="u21", name="u21")
            nc.sync.dma_start(u21_sb[:], u21[:])
            awT_sb = cpool.tile([54, C], F32, tag="awT", name="awT")
            nc.sync.dma_start(awT_sb[:], awT[:])
            ones21 = cpool.tile([1, C], F32, tag="ones21", name="ones21")
            nc.gpsimd.memset(ones21[:], 1.0)
            invnb = cpool.tile([C, COLS], BF16, tag="invnb", name="invnb")

            # Q staged for the DVE 32x32 transpose: rows 21:32 stay zero.
            q32 = cpool.tile([32, COLS], BF16, tag="q32", name="q32")
            nc.gpsimd.memset(q32[:], 0.0)

            E_res = cpool.tile([128, NB, COLS], FP8, tag="eres", name="eres")

            # ---------------- setup: E_b = exp(d2) in fp8 ----------------
            vb7_sb = ipool.tile([7, COLS], F32R, tag="vb7_Ty1", name="vb7")
            nc.sync.dma_start(vb7_sb[:], vb7[:])
            for b in range(NB):
                if b % 2 == 0:
                    ubc = stpool.tile([7, 256], F32R, tag="ubc", bufs=2,
                                      name=f"ubc_{b}")
                    nc.sync.dma_start(
                        ubc[:, 0:min(256, (NB - b) * 128)],
                        ub7[:, b * 128: min(N, (b + 2) * 128)],
                    )
                ps = pspool.tile([128, 2048], F32, tag=("psA" if b % 2 == 0 else "psB"),
                                 name=f"d2_{b}")
                for ci, (c0, cw) in enumerate(CTS):
                    mm(ps[:, ci * 512: ci * 512 + cw],
                       ubc[:, (b % 2) * 128: (b % 2 + 1) * 128],
                       vb7_sb[:, c0: c0 + cw], start=True, stop=True)
                nc.scalar.activation(E_res[:, b, :], ps[:, 0:COLS], EXPF)

            # ---------------- iterations ----------------
            qt_full = None
            for it in range(NITER):
                # ---- load Q (block-major for bilateral; y-major for spatial)
                qblk = ipool.tile([128, NB * C], BF16, tag="qblk_smx",
                                  name=f"qblk_{it}")
                if it == 0:
                    nc.sync.dma_start(qblk[:], qblk0[:])
                else:
                    nc.sync.dma_start(
                        qblk[:].rearrange("r (b c) -> r b c", b=NB),
                        qt_full[:].rearrange("(b r) c -> r b c", r=128),
                    )

                # ---- softmax, block-major
                eqB = ipool.tile([128, NB * C], BF16, tag="eqB_so54", name=f"eqB_{it}")
                nc.scalar.activation(eqB[:], qblk[:], EXPF)
                sums = ipool.tile([128, NB], F32, tag="sums", name=f"sums_{it}")
                nc.vector.reduce_sum(
                    sums[:], eqB[:].rearrange("p (b c) -> p b c", b=NB),
                    axis=mybir.AxisListType.X,
                )
                rsum = ipool.tile([128, NB], F32, tag="rsum", name=f"rsum_{it}")
                nc.vector.reciprocal(rsum[:], sums[:])
                smB = ipool.tile([128, NB, 33], BF16, tag="smB", name=f"smB_{it}")
                nc.gpsimd.memset(smB[:, :, C: 33], 1.0)
                nc.vector.tensor_mul(
                    smB[:, :, 0:C],
                    eqB[:].rearrange("p (b c) -> p b c", b=NB),
                    rsum[:].broadcast_to([128, NB, C]),
                )

                # ---- spatial input (overlaps the bilateral matmul)
                # softmax bounced to pixel-major DRAM, reloaded x-major
                smx = ipool.tile([W, H * C], BF16, tag="qblk_smx", name=f"smx_{it}")
                if it == 0:
                    # first iteration: recompute softmax in x-layout from qx0
                    qx = ipool.tile([W, H * C], BF16, tag="qx_x1", name=f"qx_{it}")
                    nc.sync.dma_start(qx[:], qx0[:])
                    eqx = ipool.tile([W, H * C], BF16, tag="sp2_eqx", name=f"eqx_{it}")
                    nc.scalar.activation(eqx[:], qx[:], EXPF)
                    sums2 = ipool.tile([W, H], F32, tag="sums", name=f"sums2_{it}")
                    nc.vector.reduce_sum(
                        sums2[:], eqx[:].rearrange("p (y c) -> p y c", y=H),
                        axis=mybir.AxisListType.X,
                    )
                    rsum2 = ipool.tile([W, H], F32, tag="rsum", name=f"rsum2_{it}")
                    nc.vector.reciprocal(rsum2[:], sums2[:])
                    nc.vector.tensor_mul(
                        smx[:].rearrange("p (y c) -> p y c", y=H),
                        eqx[:].rearrange("p (y c) -> p y c", y=H),
                        rsum2[:].broadcast_to([W, H, C]),
                    )
                else:
                    sm_pm = dpool.tile([N, C], BF16, tag="smpm", bufs=2,
                                       name=f"smpm_{it}")
                    nc.sync.dma_start(
                        sm_pm[:].rearrange("(b r) c -> r b c", r=128),
                        smB[:, :, 0:C],
                    )
                    nc.sync.dma_start(
                        smx[:].rearrange("p (y c) -> p y c", y=H),
                        sm_pm[:].rearrange("(y x) c -> x y c", x=W),
                    )

                # ---- bilateral (98 blocks, bf16 lhsT x fp8 rhs) interleaved
                # with the spatial two-pass filter so its DMA bounces hide
                # under PE work.  PE program order: blocks 0:48 | x-pass |
                # blocks 48:72 | y-pass | blocks 72:98.
                psB = pspool.tile([128, 2048], F32, tag="psB", name=f"psB_{it}")
                bl_ps = pspool.tile([33, 2048], F32, tag="psA", name=f"bl_{it}")

                def bl_chunk(b0, b1):
                    for b in range(b0, b1):
                        for ci, (c0, cw) in enumerate(CTS):
                            mm(bl_ps[:, ci * 512: ci * 512 + cw],
                               smB[:, b, :], E_res[:, b, c0: c0 + cw],
                               start=(b == 0), stop=(b == NB - 1))

                bl_chunk(0, 48)

                # x-pass: T1[x', (y c)] = gxn^T @ smx  (x'-normalizer folded)
                Ty1 = ipool.tile([W, H * C], BF16, tag="vb7_Ty1", name=f"Ty1_{it}")
                for k in range(5):
                    c0 = k * 512
                    cw = min(512, H * C - c0)
                    mm(psB[0:W, (k % 2) * 512: (k % 2) * 512 + cw],
                       gxn_sb[:], smx[:, c0: c0 + cw], start=True, stop=True)
                    nc.vector.tensor_copy(
                        Ty1[:, c0: c0 + cw],
                        psB[0:W, (k % 2) * 512: (k % 2) * 512 + cw],
                    )
                # bounce to y-on-partitions
                td = dpool.tile([W, H * C], BF16, tag="td", bufs=2, name=f"td_{it}")
                nc.sync.dma_start(td[:], Ty1[:])
                Tyx = ipool.tile([H, W * C], BF16, tag="qblk_smx", name=f"Tyx_{it}")
                nc.sync.dma_start(
                    Tyx[:].rearrange("p (x c) -> p x c", x=W),
                    td[:].rearrange("x (y c) -> y x c", y=H),
                )

                bl_chunk(48, 72)

                # y-pass: sp2[k, (x' c)] = gy2n^T @ Tyx  (y'-normalizer folded)
                sp2 = ipool.tile([YPC, W * C], BF16, tag="sp2_eqx", name=f"sp2_{it}")
                for k in range(5):
                    c0 = k * 512
                    cw = min(512, W * C - c0)
                    mm(psB[0:YPC, 1024 + (k % 2) * 512: 1024 + (k % 2) * 512 + cw],
                       gy2n_sb[:], Tyx[:, c0: c0 + cw], start=True, stop=True)
                    nc.vector.tensor_copy(
                        sp2[:, c0: c0 + cw],
                        psB[0:YPC, 1024 + (k % 2) * 512: 1024 + (k % 2) * 512 + cw],
                    )
                # relayout (k, x', c) -> (c, k, x'): transposed DMA write,
                # strided readback, DVE free-dim permute
                spd = dpool.tile([W * C, YPC], BF16, tag="spd", bufs=2,
                                 name=f"spd_{it}")
                nc.sync.dma_start(spd[:].rearrange("r k -> k r"), sp2[:])
                x1 = ipool.tile([C, W, YPC], BF16, tag="qx_x1", name=f"x1_{it}")
                nc.sync.dma_start(
                    x1[:], spd[:].rearrange("(x c) k -> c x k", x=W)
                )

                bl_chunk(72, NB)

                # ---- iteration 0: invnb = 1/nb broadcast across class rows
                if it == 0:
                    rnb = ipool.tile([1, COLS], F32, tag="qT_qfin", name="rnb")
                    nc.vector.reciprocal(rnb[:], bl_ps[32:33, 0:COLS])
                    for ci, (c0, cw) in enumerate(CTS):
                        mm(psB[0:C, ci * 512: ci * 512 + cw],
                           ones21[:], rnb[:, c0: c0 + cw], start=True, stop=True)
                    nc.vector.tensor_copy(invnb[:], psB[0:C, 0:COLS])

                # ---- stacked rhs: rows 0:21 bilateral, 21:42 spatial
                so54 = ipool.tile([54, COLS], F32, tag="eqB_so54",
                                  name=f"so54_{it}")
                nc.gpsimd.memset(so54[0:32, :], 0.0)
                nc.vector.tensor_mul(so54[0:C, :], bl_ps[0:C, 0:COLS], invnb[:])
                nc.vector.tensor_copy(
                    so54[32: 32 + C, :].rearrange("c (k x) -> c k x", k=YPC),
                    x1[:].rearrange("c x k -> c k x"),
                )

                # ---- Q = u + [A_b ; A_s]^T @ so42
                q_ps = pspool.tile([C, 2048], F32, tag="psA", name=f"qps_{it}")
                for ci, (c0, cw) in enumerate(CTS):
                    mm(q_ps[:, ci * 512: ci * 512 + cw],
                       awT_sb[:], so54[:, c0: c0 + cw], start=True, stop=True)

                if it < NITER - 1:
                    nc.vector.tensor_add(q32[0:C, :], q_ps[:, 0:COLS], u21_sb[:])
                    qT = ipool.tile([32, COLS], BF16, tag="qT_qfin", name=f"qT_{it}")
                    nc.vector.transpose(qT[:], q32[:])
                    qt_sl = dpool.tile([COLS, C], BF16, tag="qtsl", bufs=2,
                                       name=f"qtsl_{it}")
                    nc.sync.dma_start(
                        qt_sl[:].rearrange("(k p) c -> p k c", p=32),
                        qT[:].rearrange("p (k c) -> p k c", c=32)[:, :, 0:C],
                    )
                    qt_full = dpool.tile([N, C], BF16, tag="qtfull", bufs=2,
                                         addr_space="Shared", name=f"qtfull_{it}")
                    nc.gpsimd.collective_compute(
                        "AllGather",
                        mybir.AluOpType.bypass,
                        replica_groups=[list(range(NCORES))],
                        ins=[qt_sl[:]],
                        outs=[qt_full[:]],
                    )
                else:
                    q_fin = ipool.tile([C, COLS], F32, tag="qT_qfin", name="qfin")
                    nc.vector.tensor_add(q_fin[:], q_ps[:, 0:COLS], u21_sb[:])
                    nc.sync.dma_start(qt_out[:], q_fin[:])

    nc.compile()
    return nc


def _host_inputs(unaries, rgb, spatial_kernel, bilateral_kernel, compatibility_matrix):
    bf = ml_dtypes.bfloat16
    u = np.transpose(np.asarray(unaries, dtype=np.float32)[0], (2, 0, 1)).reshape(C, N)
    rgbf = np.asarray(rgb, dtype=np.float32)[0].reshape(N, 3)

    yy, xx = np.meshgrid(
        np.arange(H, dtype=np.float64), np.arange(W, dtype=np.float64), indexing="ij"
    )
    pos = np.stack([xx.ravel(), yy.ravel()], axis=1)  # [N, 2] (x, y)

    fb = np.concatenate(
        [pos / THETA_ALPHA, rgbf.astype(np.float64) / THETA_BETA], axis=1
    )
    fb -= fb.mean(axis=0, keepdims=True)  # centering: reduces fp32 cancellation
    fb32 = fb.astype(np.float32)
    sq = (fb32.astype(np.float64) ** 2).sum(axis=1)
    mhalf_sq = (-0.5 * sq).astype(np.float32)

    ub7_np = np.empty((7, N), np.float32)
    ub7_np[0:5] = fb32.T
    ub7_np[5] = mhalf_sq
    ub7_np[6] = 1.0
    vb7_np = np.empty((7, N), np.float32)
    vb7_np[0:5] = fb32.T
    vb7_np[5] = 1.0
    vb7_np[6] = mhalf_sq

    d = np.arange(-(H - 1), H, dtype=np.float64)
    g1tab = np.exp(-(d * d) / (2.0 * THETA_GAMMA**2))

    def g1(dd):
        return g1tab[np.asarray(dd) + (H - 1)]

    G = g1(np.arange(W)[:, None] - np.arange(W)[None, :])  # [t, t']
    s1 = np.array([g1(np.arange(H) - t).sum() for t in range(H)])
    gxn_np = np.ascontiguousarray((G / s1[None, :]).astype(bf))  # [x, x']

    comp = np.asarray(compatibility_matrix, dtype=np.float64)
    A_s = -(comp @ np.asarray(spatial_kernel, dtype=np.float64))
    A_b = -(comp @ np.asarray(bilateral_kernel, dtype=np.float64))
    awT_np = np.zeros((54, C), np.float32)
    awT_np[0:C] = A_b.T.astype(np.float32)
    awT_np[32: 32 + C] = A_s.T.astype(np.float32)

    uT = np.ascontiguousarray(u.T)  # [N, C]
    qblk0_np = np.ascontiguousarray(
        uT.reshape(NB, 128, C).transpose(1, 0, 2).reshape(128, NB * C).astype(bf)
    )
    qx0_np = np.ascontiguousarray(
        uT.reshape(H, W, C).transpose(1, 0, 2).reshape(W, H * C).astype(bf)
    )

    in_maps = []
    for c in range(NCORES):
        sl = slice(c * COLS, (c + 1) * COLS)
        dy = np.arange(H)[:, None] - (YPC * c + np.arange(YPC))[None, :]  # [y, k]
        gy2n_np = np.ascontiguousarray(
            (g1(dy) / s1[YPC * c + np.arange(YPC)][None, :]).astype(bf)
        )
        in_maps.append(
            dict(
                ub7=ub7_np,
                vb7=np.ascontiguousarray(vb7_np[:, sl]),
                gxn=gxn_np,
                gy2n=gy2n_np,
                u21=np.ascontiguousarray(u[:, sl]),
                qblk0=qblk0_np,
                qx0=qx0_np,
                awT=awT_np,
            )
        )
    return in_maps


def run(inputs, trace=False, **spmd_kwargs):
    in_maps = _host_inputs(**inputs)
    if "nc" not in _CACHE:
        _CACHE["nc"] = _build_program()
    nc = _CACHE["nc"]
    res = run_bass_kernel_spmd(
        nc, in_maps, core_ids=list(range(NCORES)), trace=trace, **spmd_kwargs
    )
    qs = [np.asarray(res.results[c]["qt_out"]) for c in range(NCORES)]
    Q = np.concatenate(qs, axis=1)  # [C, N]
    out = Q.reshape(C, H, W).transpose(1, 2, 0)[None].astype(np.float32)
    return out, res


def kernel(unaries, rgb, spatial_kernel, bilateral_kernel, compatibility_matrix):
    out, _ = run(
        dict(
            unaries=unaries,
            rgb=rgb,
            spatial_kernel=spatial_kernel,
            bilateral_kernel=bilateral_kernel,
            compatibility_matrix=compatibility_matrix,
        )
    )
    return out


# revision 14
# speedup vs baseline: 5.2407x; 1.9149x over previous
"""CRF-RNN layer (nn_CrfRnnLayer) as a Bass/Tile SPMD kernel on 8 TRN2 NeuronCores.

Algorithm (matches reference.py):
  N = 112*112 pixels, C = 21 classes, 5 mean-field iterations:
    sm = softmax(Q, axis=classes)
    spatial_out  = (sm @ Ks) / ns      Ks[i,j] = exp(-||p_i-p_j||^2 / (2*3^2))
    bilateral_out= (sm @ Kb) / nb      Kb from (pos/160, rgb/3) features
    Q = u - comp @ (sk @ spatial_out + bk @ bilateral_out)

Design (v2):
  - Pixel columns sharded 8 ways (1568 cols/core). Bilateral kernel slice
    E_b = Kb[:, cols] is computed once on-device (fp32r d2 matmul -> ACT exp)
    and stored *fully resident in SBUF as fp8-e4m3* (98 blocks of 128 pixels,
    ~150 KB/partition) -> zero DMA traffic in the main loop.
  - Main bilateral matmul: bf16 softmax lhsT (with a ones column at row 21
    whose PSUM row yields the normalizer nb) x fp8 E blocks = 1 cycle/row.
  - Q is AllGathered in bf16 pixel-major [N, C] layout, so the block-major
    softmax input reloads with a single strided DMA (no transposes).
  - Spatial filtering is separable with the 1/ns normalizers folded into the
    bf16 Gaussian matrices host-side. The softmaxed smB bounces through DRAM
    into [y,(x c)] layout; y-pass matmul, [k,(x c)]->[x,(k c)] bounce, x-pass
    matmul, then a small relayout into the stacked Q-update rhs. The whole
    chain overlaps the bilateral matmul.
  - SBUF pressure handled by tag-sharing temporally disjoint tiles.
"""

import numpy as np
import ml_dtypes

import concourse.mybir as mybir
import concourse.tile as tile
from concourse import bacc
from concourse.bass import _add_dep_helper
from concourse.bass_utils import run_bass_kernel_spmd

H = 112
W = 112
C = 21
N = H * W
NCORES = 8
YPC = H // NCORES            # 14 image rows per core
COLS = N // NCORES           # 1568 pixels per core
NB = 98                      # contraction blocks of 128 pixels
CTS = [(0, 512), (512, 512), (1024, 512), (1536, 32)]   # col tiles of 1568
NITER = 5
THETA_ALPHA = 160.0
THETA_BETA = 3.0
THETA_GAMMA = 3.0

F32 = mybir.dt.float32
F32R = mybir.dt.float32r
BF16 = mybir.dt.bfloat16
FP8 = mybir.dt.float8e4
EXPF = mybir.ActivationFunctionType.Exp

_CACHE = {}


def _build_program():
    nc = bacc.Bacc("TRN2", target_bir_lowering=False, debug=False, num_devices=NCORES)

    # Chain every PE matmul in emission order (ordering-only deps) so the
    # scheduler keeps same-weights matmuls adjacent for LDWEIGHTS dedup.
    _mm_state = {"prev": None}

    def mm(*args, **kwargs):
        inst = nc.tensor.matmul(*args, **kwargs)
        if _mm_state["prev"] is not None:
            _add_dep_helper(inst.ins, _mm_state["prev"].ins, sync=False,
                            reason="pe emission order")
        _mm_state["prev"] = inst
        return inst

    ub7 = nc.dram_tensor("ub7", [7, N], F32R, kind="ExternalInput")
    vb7 = nc.dram_tensor("vb7", [7, COLS], F32R, kind="ExternalInput")
    gxn = nc.dram_tensor("gxn", [W, W], BF16, kind="ExternalInput")
    gy2n = nc.dram_tensor("gy2n", [H, YPC], BF16, kind="ExternalInput")
    u21 = nc.dram_tensor("u21", [C, COLS], F32, kind="ExternalInput")
    qblk0 = nc.dram_tensor("qblk0", [128, NB * C], BF16, kind="ExternalInput")
    qx0 = nc.dram_tensor("qx0", [W, H * C], BF16, kind="ExternalInput")
    awT = nc.dram_tensor("awT", [54, C], F32, kind="ExternalInput")
    qt_out = nc.dram_tensor("qt_out", [C, COLS], F32, kind="ExternalOutput")

    with tile.TileContext(nc) as tc:
        with (
            tc.tile_pool(name="const", bufs=1) as cpool,
            tc.tile_pool(name="iter", bufs=1) as ipool,
            tc.tile_pool(name="stream", bufs=1) as stpool,
            tc.tile_pool(name="psum", bufs=1, space="PSUM") as pspool,
            tc.tile_pool(name="dram", bufs=1, space="DRAM") as dpool,
        ):
            # ---------------- constants ----------------
            gxn_sb = cpool.tile([W, W], BF16, tag="gxn", name="gxn")
            nc.sync.dma_start(gxn_sb[:], gxn[:])
            gy2n_sb = cpool.tile([H, YPC], BF16, tag="gy2n", name="gy2n")
            nc.sync.dma_start(gy2n_sb[:], gy2n[:])
            u21_sb = cpool.tile([C, COLS], F32, tag="u21", name="u21")
            nc.sync.dma_start(u21_sb[:], u21[:])
            awT_sb = cpool.tile([54, C], F32, tag="awT", name="awT")
            nc.sync.dma_start(awT_sb[:], awT[:])
            ones21 = cpool.tile([1, C], F32, tag="ones21", name="ones21")
            nc.gpsimd.memset(ones21[:], 1.0)
            invnb = cpool.tile([C, COLS], BF16, tag="invnb", name="invnb")

            # Q staged for the DVE 32x32 transpose: rows 21:32 stay zero.
            q32 = cpool.tile([32, COLS], BF16, tag="q32", name="q32")
            nc.gpsimd.memset(q32[:], 0.0)

            E_res = cpool.tile([128, NB, COLS], FP8, tag="eres", name="eres")

            # ---------------- setup: E_b = exp(d2) in fp8 ----------------
            vb7_sb = ipool.tile([7, COLS], F32R, tag="qT_qfin", name="vb7")
            nc.sync.dma_start(vb7_sb[:], vb7[:])
            for b in range(NB):
                if b % 2 == 0:
                    ubc = stpool.tile([7, 256], F32R, tag="ubc", bufs=2,
                                      name=f"ubc_{b}")
                    nc.sync.dma_start(
                        ubc[:, 0:min(256, (NB - b) * 128)],
                        ub7[:, b * 128: min(N, (b + 2) * 128)],
                    )
                ps = pspool.tile([128, 2048], F32, tag=("psA" if b % 2 == 0 else "psB"),
                                 name=f"d2_{b}")
                for ci, (c0, cw) in enumerate(CTS):
                    mm(ps[:, ci * 512: ci * 512 + cw],
                       ubc[:, (b % 2) * 128: (b % 2 + 1) * 128],
                       vb7_sb[:, c0: c0 + cw], start=True, stop=True)
                nc.scalar.activation(E_res[:, b, :], ps[:, 0:COLS], EXPF)

            # ---------------- iterations ----------------
            qt_full = None
            for it in range(NITER):
                # ---- load Q (block-major for bilateral; y-major for spatial)
                qblk = ipool.tile([128, NB * C], BF16, tag="qblk_smx",
                                  name=f"qblk_{it}")
                if it == 0:
                    nc.sync.dma_start(qblk[:], qblk0[:])
                else:
                    nc.sync.dma_start(
                        qblk[:].rearrange("r (b c) -> r b c", b=NB),
                        qt_full[:].rearrange("(r b) c -> r b c", r=128),
                    )

                # ---- softmax, block-major
                eqB = ipool.tile([128, NB * C], BF16, tag="eqB_so54", name=f"eqB_{it}")
                nc.scalar.activation(eqB[:], qblk[:], EXPF)
                sums = ipool.tile([128, NB], F32, tag="sums", name=f"sums_{it}")
                nc.vector.reduce_sum(
                    sums[:], eqB[:].rearrange("p (b c) -> p b c", b=NB),
                    axis=mybir.AxisListType.X,
                )
                rsum = ipool.tile([128, NB], F32, tag="rsum", name=f"rsum_{it}")
                nc.vector.reciprocal(rsum[:], sums[:])
                smB = ipool.tile([128, NB, 33], BF16, tag="smB", name=f"smB_{it}")
                nc.gpsimd.memset(smB[:, :, C: 33], 1.0)
                nc.vector.tensor_mul(
                    smB[:, :, 0:C],
                    eqB[:].rearrange("p (b c) -> p b c", b=NB),
                    rsum[:].broadcast_to([128, NB, C]),
                )

                # ---- spatial input (overlaps the bilateral matmul)
                # softmax bounced to pixel-major DRAM, reloaded x-major
                smx = ipool.tile([W, H * C], BF16, tag="qblk_smx", name=f"smx_{it}")
                if it == 0:
                    # first iteration: recompute softmax in x-layout from qx0
                    qx = ipool.tile([W, H * C], BF16, tag="qx_spT", name=f"qx_{it}")
                    nc.sync.dma_start(qx[:], qx0[:])
                    eqx = ipool.tile([W, H * C], BF16, tag="sp2p", name=f"eqx_{it}")
                    nc.scalar.activation(eqx[:], qx[:], EXPF)
                    sums2 = ipool.tile([W, H], F32, tag="sums", name=f"sums2_{it}")
                    nc.vector.reduce_sum(
                        sums2[:], eqx[:].rearrange("p (y c) -> p y c", y=H),
                        axis=mybir.AxisListType.X,
                    )
                    rsum2 = ipool.tile([W, H], F32, tag="rsum", name=f"rsum2_{it}")
                    nc.vector.reciprocal(rsum2[:], sums2[:])
                    nc.vector.tensor_mul(
                        smx[:].rearrange("p (y c) -> p y c", y=H),
                        eqx[:].rearrange("p (y c) -> p y c", y=H),
                        rsum2[:].broadcast_to([W, H, C]),
                    )
                else:
                    sm_pm = dpool.tile([N, C], BF16, tag="smpm", bufs=2,
                                       name=f"smpm_{it}")
                    nc.sync.dma_start(
                        sm_pm[:].rearrange("(r b) c -> r b c", r=128),
                        smB[:, :, 0:C],
                    )
                    nc.sync.dma_start(
                        smx[:].rearrange("p (y c) -> p y c", y=H),
                        sm_pm[:].rearrange("(y x) c -> x y c", x=W),
                    )

                # ---- bilateral (98 blocks, bf16 lhsT x fp8 rhs) interleaved
                # with the spatial two-pass filter so its DMA bounces hide
                # under PE work.  PE program order: blocks 0:48 | x-pass |
                # blocks 48:72 | y-pass | blocks 72:98.
                psB = pspool.tile([128, 2048], F32, tag="psB", name=f"psB_{it}")
                bl_ps = pspool.tile([33, 2048], F32, tag="psA", name=f"bl_{it}")

                def bl_chunk(b0, b1):
                    for b in range(b0, b1):
                        for ci, (c0, cw) in enumerate(CTS):
                            mm(bl_ps[:, ci * 512: ci * 512 + cw],
                               smB[:, b, :], E_res[:, b, c0: c0 + cw],
                               start=(b == 0), stop=(b == NB - 1))

                bl_chunk(0, 48)

                # x-pass: T1[x', (y c)] = gxn^T @ smx  (x'-normalizer folded)
                Ty1 = ipool.tile([W, H * C], BF16, tag="Ty1", name=f"Ty1_{it}")
                for k in range(5):
                    c0 = k * 512
                    cw = min(512, H * C - c0)
                    mm(psB[0:W, (k % 2) * 512: (k % 2) * 512 + cw],
                       gxn_sb[:], smx[:, c0: c0 + cw], start=True, stop=True)
                    nc.vector.tensor_copy(
                        Ty1[:, c0: c0 + cw],
                        psB[0:W, (k % 2) * 512: (k % 2) * 512 + cw],
                    )
                # bounce to y-on-partitions
                td = dpool.tile([W, H * C], BF16, tag="td", bufs=2, name=f"td_{it}")
                nc.sync.dma_start(td[:], Ty1[:])
                Tyx = ipool.tile([H, W * C], BF16, tag="qblk_smx", name=f"Tyx_{it}")
                nc.sync.dma_start(
                    Tyx[:].rearrange("p (x c) -> p x c", x=W),
                    td[:].rearrange("x (y c) -> y x c", y=H),
                )

                bl_chunk(48, 72)

                # y-pass: sp2p[k, x', c32] = gy2n^T @ Tyx, class dim padded
                # to 32 so a DVE 32x32 stream transpose yields c-partitions
                sp2p = ipool.tile([32, W * 32], BF16, tag="sp2p", name=f"sp2p_{it}")
                nc.gpsimd.memset(sp2p[:], 0.0)
                XCH = [(0, 24), (24, 24), (48, 24), (72, 24), (96, 16)]
                for k, (x0, xw) in enumerate(XCH):
                    mm(psB[0:YPC, 1024 + (k % 2) * 512: 1024 + (k % 2) * 512 + xw * C],
                       gy2n_sb[:], Tyx[:, x0 * C: (x0 + xw) * C],
                       start=True, stop=True)
                    nc.vector.tensor_copy(
                        sp2p[:].rearrange("p (x c) -> p x c", c=32)[0:YPC, x0: x0 + xw, 0:C],
                        psB[0:YPC, 1024 + (k % 2) * 512: 1024 + (k % 2) * 512 + xw * C]
                        .rearrange("p (x c) -> p x c", c=C),
                    )
                spT = ipool.tile([32, W * 32], BF16, tag="qx_spT", name=f"spT_{it}")
                nc.vector.transpose(spT[:], sp2p[:])

                bl_chunk(72, NB)

                # ---- iteration 0: invnb = 1/nb broadcast across class rows
                if it == 0:
                    rnb = ipool.tile([1, COLS], F32, tag="qT_qfin", name="rnb")
                    nc.vector.reciprocal(rnb[:], bl_ps[32:33, 0:COLS])
                    for ci, (c0, cw) in enumerate(CTS):
                        mm(psB[0:C, ci * 512: ci * 512 + cw],
                           ones21[:], rnb[:, c0: c0 + cw], start=True, stop=True)
                    nc.vector.tensor_copy(invnb[:], psB[0:C, 0:COLS])

                # ---- stacked rhs: rows 0:21 bilateral, 21:42 spatial
                so54 = ipool.tile([54, COLS], F32, tag="eqB_so54",
                                  name=f"so54_{it}")
                nc.gpsimd.memset(so54[0:32, :], 0.0)
                nc.vector.tensor_mul(so54[0:C, :], bl_ps[0:C, 0:COLS], invnb[:])
                nc.vector.tensor_copy(
                    so54[32: 32 + C, :].rearrange("c (k x) -> c k x", k=YPC),
                    spT[:].rearrange("p (x c) -> p x c", c=32)[0:C, :, 0:YPC]
                    .rearrange("c x k -> c k x"),
                )

                # ---- Q = u + [A_b ; A_s]^T @ so42
                q_ps = pspool.tile([C, 2048], F32, tag="psA", name=f"qps_{it}")
                for ci, (c0, cw) in enumerate(CTS):
                    mm(q_ps[:, ci * 512: ci * 512 + cw],
                       awT_sb[:], so54[:, c0: c0 + cw], start=True, stop=True)

                if it < NITER - 1:
                    nc.vector.tensor_add(q32[0:C, :], q_ps[:, 0:COLS], u21_sb[:])
                    qT = ipool.tile([32, COLS], BF16, tag="qT_qfin", name=f"qT_{it}")
                    nc.vector.transpose(qT[:], q32[:])
                    qt_sl = dpool.tile([COLS, C], BF16, tag="qtsl", bufs=2,
                                       name=f"qtsl_{it}")
                    nc.sync.dma_start(
                        qt_sl[:].rearrange("(k p) c -> p k c", p=32),
                        qT[:].rearrange("p (k c) -> p k c", c=32)[:, :, 0:C],
                    )
                    qt_full = dpool.tile([N, C], BF16, tag="qtfull", bufs=2,
                                         addr_space="Shared", name=f"qtfull_{it}")
                    nc.gpsimd.collective_compute(
                        "AllGather",
                        mybir.AluOpType.bypass,
                        replica_groups=[list(range(NCORES))],
                        ins=[qt_sl[:]],
                        outs=[qt_full[:]],
                    )
                else:
                    q_fin = ipool.tile([C, COLS], F32, tag="qT_qfin", name="qfin")
                    nc.vector.tensor_add(q_fin[:], q_ps[:, 0:COLS], u21_sb[:])
                    nc.sync.dma_start(qt_out[:], q_fin[:])

    nc.compile()
    return nc


def _host_inputs(unaries, rgb, spatial_kernel, bilateral_kernel, compatibility_matrix):
    bf = ml_dtypes.bfloat16
    u = np.transpose(np.asarray(unaries, dtype=np.float32)[0], (2, 0, 1)).reshape(C, N)
    rgbf = np.asarray(rgb, dtype=np.float32)[0].reshape(N, 3)

    yy, xx = np.meshgrid(
        np.arange(H, dtype=np.float64), np.arange(W, dtype=np.float64), indexing="ij"
    )
    pos = np.stack([xx.ravel(), yy.ravel()], axis=1)  # [N, 2] (x, y)

    fb = np.concatenate(
        [pos / THETA_ALPHA, rgbf.astype(np.float64) / THETA_BETA], axis=1
    )
    fb -= fb.mean(axis=0, keepdims=True)  # centering: reduces fp32 cancellation
    fb32 = fb.astype(np.float32)
    sq = (fb32.astype(np.float64) ** 2).sum(axis=1)
    mhalf_sq = (-0.5 * sq).astype(np.float32)

    ub7_np = np.empty((7, N), np.float32)
    ub7_np[0:5] = fb32.T
    ub7_np[5] = mhalf_sq
    ub7_np[6] = 1.0
    vb7_np = np.empty((7, N), np.float32)
    vb7_np[0:5] = fb32.T
    vb7_np[5] = 1.0
    vb7_np[6] = mhalf_sq

    d = np.arange(-(H - 1), H, dtype=np.float64)
    g1tab = np.exp(-(d * d) / (2.0 * THETA_GAMMA**2))

    def g1(dd):
        return g1tab[np.asarray(dd) + (H - 1)]

    G = g1(np.arange(W)[:, None] - np.arange(W)[None, :])  # [t, t']
    s1 = np.array([g1(np.arange(H) - t).sum() for t in range(H)])
    gxn_np = np.ascontiguousarray((G / s1[None, :]).astype(bf))  # [x, x']

    comp = np.asarray(compatibility_matrix, dtype=np.float64)
    A_s = -(comp @ np.asarray(spatial_kernel, dtype=np.float64))
    A_b = -(comp @ np.asarray(bilateral_kernel, dtype=np.float64))
    awT_np = np.zeros((54, C), np.float32)
    awT_np[0:C] = A_b.T.astype(np.float32)
    awT_np[32: 32 + C] = A_s.T.astype(np.float32)

    uT = np.ascontiguousarray(u.T)  # [N, C]
    # contraction block b holds pixels {p*98+b}; DMA column (b,p) = pixel 98p+b
    X = (98 * np.arange(128)[None, :] + np.arange(NB)[:, None]).reshape(-1)
    ub7_np = np.ascontiguousarray(ub7_np[:, X])
    qblk0_np = np.ascontiguousarray(
        uT.reshape(128, NB, C).reshape(128, NB * C).astype(bf)
    )
    qx0_np = np.ascontiguousarray(
        uT.reshape(H, W, C).transpose(1, 0, 2).reshape(W, H * C).astype(bf)
    )

    in_maps = []
    for c in range(NCORES):
        sl = slice(c * COLS, (c + 1) * COLS)
        dy = np.arange(H)[:, None] - (YPC * c + np.arange(YPC))[None, :]  # [y, k]
        gy2n_np = np.ascontiguousarray(
            (g1(dy) / s1[YPC * c + np.arange(YPC)][None, :]).astype(bf)
        )
        in_maps.append(
            dict(
                ub7=ub7_np,
                vb7=np.ascontiguousarray(vb7_np[:, sl]),
                gxn=gxn_np,
                gy2n=gy2n_np,
                u21=np.ascontiguousarray(u[:, sl]),
                qblk0=qblk0_np,
                qx0=qx0_np,
                awT=awT_np,
            )
        )
    return in_maps


def run(inputs, trace=False, **spmd_kwargs):
    in_maps = _host_inputs(**inputs)
    if "nc" not in _CACHE:
        _CACHE["nc"] = _build_program()
    nc = _CACHE["nc"]
    res = run_bass_kernel_spmd(
        nc, in_maps, core_ids=list(range(NCORES)), trace=trace, **spmd_kwargs
    )
    qs = [np.asarray(res.results[c]["qt_out"]) for c in range(NCORES)]
    Q = np.concatenate(qs, axis=1)  # [C, N]
    out = Q.reshape(C, H, W).transpose(1, 2, 0)[None].astype(np.float32)
    return out, res


def kernel(unaries, rgb, spatial_kernel, bilateral_kernel, compatibility_matrix):
    out, _ = run(
        dict(
            unaries=unaries,
            rgb=rgb,
            spatial_kernel=spatial_kernel,
            bilateral_kernel=bilateral_kernel,
            compatibility_matrix=compatibility_matrix,
        )
    )
    return out


# revision 15
# speedup vs baseline: 5.8076x; 1.1082x over previous
"""CRF-RNN layer (nn_CrfRnnLayer) as a Bass/Tile SPMD kernel on 8 TRN2 NeuronCores.

Algorithm (matches reference.py):
  N = 112*112 pixels, C = 21 classes, 5 mean-field iterations:
    sm = softmax(Q, axis=classes)
    spatial_out  = (sm @ Ks) / ns      Ks[i,j] = exp(-||p_i-p_j||^2 / (2*3^2))
    bilateral_out= (sm @ Kb) / nb      Kb from (pos/160, rgb/3) features
    Q = u - comp @ (sk @ spatial_out + bk @ bilateral_out)

Design (v2):
  - Pixel columns sharded 8 ways (1568 cols/core). Bilateral kernel slice
    E_b = Kb[:, cols] is computed once on-device (fp32r d2 matmul -> ACT exp)
    and stored *fully resident in SBUF as fp8-e4m3* (98 blocks of 128 pixels,
    ~150 KB/partition) -> zero DMA traffic in the main loop.
  - Main bilateral matmul: bf16 softmax lhsT (with a ones column at row 21
    whose PSUM row yields the normalizer nb) x fp8 E blocks = 1 cycle/row.
  - Q is AllGathered in bf16 pixel-major [N, C] layout, so the block-major
    softmax input reloads with a single strided DMA (no transposes).
  - Spatial filtering is separable with the 1/ns normalizers folded into the
    bf16 Gaussian matrices host-side. The softmaxed smB bounces through DRAM
    into [y,(x c)] layout; y-pass matmul, [k,(x c)]->[x,(k c)] bounce, x-pass
    matmul, then a small relayout into the stacked Q-update rhs. The whole
    chain overlaps the bilateral matmul.
  - SBUF pressure handled by tag-sharing temporally disjoint tiles.
"""

import numpy as np
import ml_dtypes

import concourse.mybir as mybir
import concourse.tile as tile
from concourse import bacc
from concourse.bass import _add_dep_helper
from concourse.bass_utils import run_bass_kernel_spmd

H = 112
W = 112
C = 21
N = H * W
NCORES = 8
YPC = H // NCORES            # 14 image rows per core
COLS = N // NCORES           # 1568 pixels per core
NB = 98                      # contraction blocks of 128 pixels
CTS = [(0, 512), (512, 512), (1024, 512), (1536, 32)]   # col tiles of 1568
NITER = 5
THETA_ALPHA = 160.0
THETA_BETA = 3.0
THETA_GAMMA = 3.0

F32 = mybir.dt.float32
F32R = mybir.dt.float32r
BF16 = mybir.dt.bfloat16
FP8 = mybir.dt.float8e4
EXPF = mybir.ActivationFunctionType.Exp

_CACHE = {}


def _build_program():
    nc = bacc.Bacc("TRN2", target_bir_lowering=False, debug=False, num_devices=NCORES)

    # Chain every PE matmul in emission order (ordering-only deps) so the
    # scheduler keeps same-weights matmuls adjacent for LDWEIGHTS dedup.
    _mm_state = {"prev": None}

    def mm(*args, **kwargs):
        inst = nc.tensor.matmul(*args, **kwargs)
        if _mm_state["prev"] is not None:
            _add_dep_helper(inst.ins, _mm_state["prev"].ins, sync=False,
                            reason="pe emission order")
        _mm_state["prev"] = inst
        return inst

    ub7 = nc.dram_tensor("ub7", [7, N], F32R, kind="ExternalInput")
    vb7 = nc.dram_tensor("vb7", [7, COLS], F32R, kind="ExternalInput")
    gxn = nc.dram_tensor("gxn", [W, W], BF16, kind="ExternalInput")
    gy2n = nc.dram_tensor("gy2n", [H, YPC], BF16, kind="ExternalInput")
    u21 = nc.dram_tensor("u21", [C, COLS], F32, kind="ExternalInput")
    qblk0 = nc.dram_tensor("qblk0", [128, NB * C], BF16, kind="ExternalInput")
    qy0 = nc.dram_tensor("qy0", [H, W * C], BF16, kind="ExternalInput")
    awT = nc.dram_tensor("awT", [54, C], F32, kind="ExternalInput")
    qt_out = nc.dram_tensor("qt_out", [C, COLS], F32, kind="ExternalOutput")

    with tile.TileContext(nc) as tc:
        with (
            tc.tile_pool(name="const", bufs=1) as cpool,
            tc.tile_pool(name="iter", bufs=1) as ipool,
            tc.tile_pool(name="stream", bufs=1) as stpool,
            tc.tile_pool(name="psum", bufs=1, space="PSUM") as pspool,
            tc.tile_pool(name="dram", bufs=1, space="DRAM") as dpool,
        ):
            # ---------------- constants ----------------
            gxn_sb = cpool.tile([W, W], BF16, tag="gxn", name="gxn")
            nc.sync.dma_start(gxn_sb[:], gxn[:])
            gy2n_sb = cpool.tile([H, YPC], BF16, tag="gy2n", name="gy2n")
            nc.sync.dma_start(gy2n_sb[:], gy2n[:])
            u21_sb = cpool.tile([C, COLS], F32, tag="u21", name="u21")
            nc.sync.dma_start(u21_sb[:], u21[:])
            awT_sb = cpool.tile([54, C], F32, tag="awT", name="awT")
            nc.sync.dma_start(awT_sb[:], awT[:])
            ones21 = cpool.tile([1, C], F32, tag="ones21", name="ones21")
            nc.gpsimd.memset(ones21[:], 1.0)
            invnb = cpool.tile([C, COLS], BF16, tag="invnb", name="invnb")

            # Q staged for the DVE 32x32 transpose: rows 21:32 stay zero.
            q32 = cpool.tile([32, COLS], BF16, tag="q32", name="q32")
            nc.gpsimd.memset(q32[:], 0.0)

            E_res = cpool.tile([128, NB, COLS], FP8, tag="eres", name="eres")

            qt_full = None

            def head(it):
                """Load Q, softmax block-major (bilateral lhsT) + y-major
                (spatial input). Returns (smB, smy)."""
                qblk = ipool.tile([128, NB * C], BF16, tag="qblk_smy",
                                  name=f"qblk_{it}")
                if it == 0:
                    nc.sync.dma_start(qblk[:], qblk0[:])
                else:
                    nc.sync.dma_start(
                        qblk[:].rearrange("r (b c) -> r b c", b=NB),
                        qt_full[:].rearrange("(r b) c -> r b c", r=128),
                    )
                eqB = ipool.tile([128, NB * C], BF16, tag="eqB_so54",
                                 name=f"eqB_{it}")
                nc.scalar.activation(eqB[:], qblk[:], EXPF)
                sums = ipool.tile([128, NB], F32, tag="sums", name=f"sums_{it}")
                nc.vector.reduce_sum(
                    sums[:], eqB[:].rearrange("p (b c) -> p b c", b=NB),
                    axis=mybir.AxisListType.X,
                )
                rsum = ipool.tile([128, NB], F32, tag="rsum", name=f"rsum_{it}")
                nc.vector.reciprocal(rsum[:], sums[:])
                smB = ipool.tile([128, NB, 33], BF16, tag="smB", name=f"smB_{it}")
                nc.gpsimd.memset(smB[:, :, C: 33], 1.0)
                nc.vector.tensor_mul(
                    smB[:, :, 0:C],
                    eqB[:].rearrange("p (b c) -> p b c", b=NB),
                    rsum[:].broadcast_to([128, NB, C]),
                )

                smy = ipool.tile([H, W * C], BF16, tag="qblk_smy",
                                 name=f"smy_{it}")
                if it == 0:
                    qy = ipool.tile([H, W * C], BF16, tag="qy_spc",
                                    name=f"qy_{it}")
                    nc.sync.dma_start(qy[:], qy0[:])
                    eqy = ipool.tile([H, W * C], BF16, tag="eqy_spT",
                                     name=f"eqy_{it}")
                    nc.scalar.activation(eqy[:], qy[:], EXPF)
                    sums2 = ipool.tile([H, W], F32, tag="sums", name=f"sums2_{it}")
                    nc.vector.reduce_sum(
                        sums2[:], eqy[:].rearrange("p (x c) -> p x c", x=W),
                        axis=mybir.AxisListType.X,
                    )
                    rsum2 = ipool.tile([H, W], F32, tag="rsum", name=f"rsum2_{it}")
                    nc.vector.reciprocal(rsum2[:], sums2[:])
                    nc.vector.tensor_mul(
                        smy[:].rearrange("p (x c) -> p x c", x=W),
                        eqy[:].rearrange("p (x c) -> p x c", x=W),
                        rsum2[:].broadcast_to([H, W, C]),
                    )
                else:
                    sm_pm = dpool.tile([N, C], BF16, tag="smpm", bufs=2,
                                       name=f"smpm_{it}")
                    nc.sync.dma_start(
                        sm_pm[:].rearrange("(r b) c -> r b c", r=128),
                        smB[:, :, 0:C],
                    )
                    nc.sync.dma_start(
                        smy[:], sm_pm[:].rearrange("(y f) c -> y (f c)", y=H)
                    )
                return smB, smy

            # it0 head first: its ACT exp beats the 98 setup exps into the
            # strict-FIFO scalar queue, so iteration 0 starts right after
            # the d2 matmuls finish.
            head0 = head(0)

            # ---------------- setup: E_b = exp(d2) in fp8 ----------------
            vb7_sb = ipool.tile([7, COLS], F32R, tag="qT_qfin", name="vb7")
            nc.sync.dma_start(vb7_sb[:], vb7[:])
            for b in range(NB):
                if b % 2 == 0:
                    ubc = stpool.tile([7, 256], F32R, tag="ubc", bufs=2,
                                      name=f"ubc_{b}")
                    nc.sync.dma_start(
                        ubc[:, 0:min(256, (NB - b) * 128)],
                        ub7[:, b * 128: min(N, (b + 2) * 128)],
                    )
                ps = pspool.tile([128, 2048], F32, tag=("psA" if b % 2 == 0 else "psB"),
                                 name=f"d2_{b}")
                for ci, (c0, cw) in enumerate(CTS):
                    mm(ps[:, ci * 512: ci * 512 + cw],
                       ubc[:, (b % 2) * 128: (b % 2 + 1) * 128],
                       vb7_sb[:, c0: c0 + cw], start=True, stop=True)
                nc.scalar.activation(E_res[:, b, :], ps[:, 0:COLS], EXPF)

            # ---------------- iterations ----------------
            for it in range(NITER):
                smB, smy = head0 if it == 0 else head(it)

                psB = pspool.tile([128, 2048], F32, tag="psB", name=f"psB_{it}")
                bl_ps = pspool.tile([33, 2048], F32, tag="psA", name=f"bl_{it}")

                def bl_chunk(b0, b1):
                    for b in range(b0, b1):
                        for ci, (c0, cw) in enumerate(CTS):
                            mm(bl_ps[:, ci * 512: ci * 512 + cw],
                               smB[:, b, :], E_res[:, b, c0: c0 + cw],
                               start=(b == 0), stop=(b == NB - 1))

                bl_chunk(0, 56)

                # y-pass: Tk[k, (x c)] = gy2n^T @ smy  (y'-normalizer folded)
                Tk = ipool.tile([YPC, W * C], BF16, tag="Tk", name=f"Tk_{it}")
                XCH = [(0, 24), (24, 24), (48, 24), (72, 24), (96, 16)]
                for k, (x0, xw) in enumerate(XCH):
                    mm(psB[0:YPC, (k % 2) * 512: (k % 2) * 512 + xw * C],
                       gy2n_sb[:], smy[:, x0 * C: (x0 + xw) * C],
                       start=True, stop=True)
                    nc.vector.tensor_copy(
                        Tk[:, x0 * C: (x0 + xw) * C],
                        psB[0:YPC, (k % 2) * 512: (k % 2) * 512 + xw * C],
                    )
                # bounce to x-on-partitions (33K elems only)
                td = dpool.tile([YPC, W * C], BF16, tag="td", bufs=2, name=f"td_{it}")
                nc.sync.dma_start(td[:], Tk[:])
                Txk = ipool.tile([W, YPC * C], BF16, tag="Txk", name=f"Txk_{it}")
                nc.sync.dma_start(
                    Txk[:].rearrange("x (k c) -> x k c", k=YPC),
                    td[:].rearrange("k (x c) -> x k c", x=W),
                )

                bl_chunk(56, 80)

                # x-pass in 4 chunks of 28 output columns; each [28, (k c)]
                # PSUM block is padded to [32, (k, 32)] and DVE-32-transposed
                # into class partitions, then copied into the stacked rhs.
                so54 = ipool.tile([54, COLS], F32, tag="eqB_so54",
                                  name=f"so54_{it}")
                for j, x0 in enumerate([0, 28, 56, 84]):
                    pslab = psB[0:28, 1024 + (j % 2) * 512:
                                 1024 + (j % 2) * 512 + YPC * C]
                    mm(pslab, gxn_sb[:, x0: x0 + 28], Txk[:],
                       start=True, stop=True)
                    spc = ipool.tile([32, YPC * 32], BF16, tag="qy_spc",
                                     name=f"spc_{it}_{j}")
                    nc.gpsimd.memset(spc[:], 0.0)
                    nc.vector.tensor_copy(
                        spc[:].rearrange("p (k c) -> p k c", c=32)[0:28, :, 0:C],
                        pslab.rearrange("p (k c) -> p k c", c=C),
                    )
                    spT = ipool.tile([32, YPC * 32], BF16, tag="eqy_spT",
                                     name=f"spT_{it}_{j}")
                    nc.vector.transpose(spT[:], spc[:])
                    nc.vector.tensor_copy(
                        so54[32: 32 + C, :]
                        .rearrange("c (k x) -> c k x", k=YPC)[:, :, x0: x0 + 28],
                        spT[:].rearrange("p (k x) -> p k x", x=32)[0:C, :, 0:28],
                    )

                bl_chunk(80, NB)

                # ---- iteration 0: invnb = 1/nb broadcast across class rows
                if it == 0:
                    rnb = ipool.tile([1, COLS], F32, tag="qT_qfin", name="rnb")
                    nc.vector.reciprocal(rnb[:], bl_ps[32:33, 0:COLS])
                    for ci, (c0, cw) in enumerate(CTS):
                        mm(psB[0:C, ci * 512: ci * 512 + cw],
                           ones21[:], rnb[:, c0: c0 + cw], start=True, stop=True)
                    nc.vector.tensor_copy(invnb[:], psB[0:C, 0:COLS])

                # ---- stacked rhs rows 0:21 bilateral (rows 21:32 zero)
                nc.gpsimd.memset(so54[0:32, :], 0.0)
                nc.vector.tensor_mul(so54[0:C, :], bl_ps[0:C, 0:COLS], invnb[:])

                # ---- Q = u + [A_b ; A_s]^T @ so54
                q_ps = pspool.tile([C, 2048], F32, tag="psA", name=f"qps_{it}")
                for ci, (c0, cw) in enumerate(CTS):
                    mm(q_ps[:, ci * 512: ci * 512 + cw],
                       awT_sb[:], so54[:, c0: c0 + cw], start=True, stop=True)

                if it < NITER - 1:
                    nc.vector.tensor_add(q32[0:C, :], q_ps[:, 0:COLS], u21_sb[:])
                    qT = ipool.tile([32, COLS], BF16, tag="qT_qfin", name=f"qT_{it}")
                    nc.vector.transpose(qT[:], q32[:])
                    qt_sl = dpool.tile([COLS, C], BF16, tag="qtsl", bufs=2,
                                       name=f"qtsl_{it}")
                    nc.sync.dma_start(
                        qt_sl[:].rearrange("(k p) c -> p k c", p=32),
                        qT[:].rearrange("p (k c) -> p k c", c=32)[:, :, 0:C],
                    )
                    qt_full = dpool.tile([N, C], BF16, tag="qtfull", bufs=2,
                                         addr_space="Shared", name=f"qtfull_{it}")
                    nc.gpsimd.collective_compute(
                        "AllGather",
                        mybir.AluOpType.bypass,
                        replica_groups=[list(range(NCORES))],
                        ins=[qt_sl[:]],
                        outs=[qt_full[:]],
                    )
                else:
                    q_fin = ipool.tile([C, COLS], F32, tag="qT_qfin", name="qfin")
                    nc.vector.tensor_add(q_fin[:], q_ps[:, 0:COLS], u21_sb[:])
                    nc.sync.dma_start(qt_out[:], q_fin[:])

    nc.compile()
    return nc


def _host_inputs(unaries, rgb, spatial_kernel, bilateral_kernel, compatibility_matrix):
    bf = ml_dtypes.bfloat16
    u = np.transpose(np.asarray(unaries, dtype=np.float32)[0], (2, 0, 1)).reshape(C, N)
    rgbf = np.asarray(rgb, dtype=np.float32)[0].reshape(N, 3)

    yy, xx = np.meshgrid(
        np.arange(H, dtype=np.float64), np.arange(W, dtype=np.float64), indexing="ij"
    )
    pos = np.stack([xx.ravel(), yy.ravel()], axis=1)  # [N, 2] (x, y)

    fb = np.concatenate(
        [pos / THETA_ALPHA, rgbf.astype(np.float64) / THETA_BETA], axis=1
    )
    fb -= fb.mean(axis=0, keepdims=True)  # centering: reduces fp32 cancellation
    fb32 = fb.astype(np.float32)
    sq = (fb32.astype(np.float64) ** 2).sum(axis=1)
    mhalf_sq = (-0.5 * sq).astype(np.float32)

    ub7_np = np.empty((7, N), np.float32)
    ub7_np[0:5] = fb32.T
    ub7_np[5] = mhalf_sq
    ub7_np[6] = 1.0
    vb7_np = np.empty((7, N), np.float32)
    vb7_np[0:5] = fb32.T
    vb7_np[5] = 1.0
    vb7_np[6] = mhalf_sq

    d = np.arange(-(H - 1), H, dtype=np.float64)
    g1tab = np.exp(-(d * d) / (2.0 * THETA_GAMMA**2))

    def g1(dd):
        return g1tab[np.asarray(dd) + (H - 1)]

    G = g1(np.arange(W)[:, None] - np.arange(W)[None, :])  # [t, t']
    s1 = np.array([g1(np.arange(H) - t).sum() for t in range(H)])
    gxn_np = np.ascontiguousarray((G / s1[None, :]).astype(bf))  # [x, x']

    comp = np.asarray(compatibility_matrix, dtype=np.float64)
    A_s = -(comp @ np.asarray(spatial_kernel, dtype=np.float64))
    A_b = -(comp @ np.asarray(bilateral_kernel, dtype=np.float64))
    awT_np = np.zeros((54, C), np.float32)
    awT_np[0:C] = A_b.T.astype(np.float32)
    awT_np[32: 32 + C] = A_s.T.astype(np.float32)

    uT = np.ascontiguousarray(u.T)  # [N, C]
    # contraction block b holds pixels {p*98+b}; DMA column (b,p) = pixel 98p+b
    X = (98 * np.arange(128)[None, :] + np.arange(NB)[:, None]).reshape(-1)
    ub7_np = np.ascontiguousarray(ub7_np[:, X])
    qblk0_np = np.ascontiguousarray(
        uT.reshape(128, NB, C).reshape(128, NB * C).astype(bf)
    )
    qy0_np = np.ascontiguousarray(uT.reshape(H, W * C).astype(bf))

    in_maps = []
    for c in range(NCORES):
        sl = slice(c * COLS, (c + 1) * COLS)
        dy = np.arange(H)[:, None] - (YPC * c + np.arange(YPC))[None, :]  # [y, k]
        gy2n_np = np.ascontiguousarray(
            (g1(dy) / s1[YPC * c + np.arange(YPC)][None, :]).astype(bf)
        )
        in_maps.append(
            dict(
                ub7=ub7_np,
                vb7=np.ascontiguousarray(vb7_np[:, sl]),
                gxn=gxn_np,
                gy2n=gy2n_np,
                u21=np.ascontiguousarray(u[:, sl]),
                qblk0=qblk0_np,
                qy0=qy0_np,
                awT=awT_np,
            )
        )
    return in_maps


def run(inputs, trace=False, **spmd_kwargs):
    in_maps = _host_inputs(**inputs)
    if "nc" not in _CACHE:
        _CACHE["nc"] = _build_program()
    nc = _CACHE["nc"]
    res = run_bass_kernel_spmd(
        nc, in_maps, core_ids=list(range(NCORES)), trace=trace, **spmd_kwargs
    )
    qs = [np.asarray(res.results[c]["qt_out"]) for c in range(NCORES)]
    Q = np.concatenate(qs, axis=1)  # [C, N]
    out = Q.reshape(C, H, W).transpose(1, 2, 0)[None].astype(np.float32)
    return out, res


def kernel(unaries, rgb, spatial_kernel, bilateral_kernel, compatibility_matrix):
    out, _ = run(
        dict(
            unaries=unaries,
            rgb=rgb,
            spatial_kernel=spatial_kernel,
            bilateral_kernel=bilateral_kernel,
            compatibility_matrix=compatibility_matrix,
        )
    )
    return out


# revision 16
# speedup vs baseline: 7.2291x; 1.2448x over previous
"""CRF-RNN layer (nn_CrfRnnLayer) as a Bass/Tile SPMD kernel on 8 TRN2 NeuronCores.

Algorithm (matches reference.py):
  N = 112*112 pixels, C = 21 classes, 5 mean-field iterations:
    sm = softmax(Q, axis=classes)
    spatial_out  = (sm @ Ks) / ns      Ks[i,j] = exp(-||p_i-p_j||^2 / (2*3^2))
    bilateral_out= (sm @ Kb) / nb      Kb from (pos/160, rgb/3) features
    Q = u - comp @ (sk @ spatial_out + bk @ bilateral_out)

Design (v2):
  - Pixel columns sharded 8 ways (1568 cols/core). Bilateral kernel slice
    E_b = Kb[:, cols] is computed once on-device (fp32r d2 matmul -> ACT exp)
    and stored *fully resident in SBUF as fp8-e4m3* (98 blocks of 128 pixels,
    ~150 KB/partition) -> zero DMA traffic in the main loop.
  - Main bilateral matmul: bf16 softmax lhsT (with a ones column at row 21
    whose PSUM row yields the normalizer nb) x fp8 E blocks = 1 cycle/row.
  - Q is AllGathered in bf16 pixel-major [N, C] layout, so the block-major
    softmax input reloads with a single strided DMA (no transposes).
  - Spatial filtering is separable with the 1/ns normalizers folded into the
    bf16 Gaussian matrices host-side. The softmaxed smB bounces through DRAM
    into [y,(x c)] layout; y-pass matmul, [k,(x c)]->[x,(k c)] bounce, x-pass
    matmul, then a small relayout into the stacked Q-update rhs. The whole
    chain overlaps the bilateral matmul.
  - SBUF pressure handled by tag-sharing temporally disjoint tiles.
"""

import numpy as np
import ml_dtypes

import concourse.mybir as mybir
import concourse.tile as tile
from concourse import bacc
from concourse.bass import _add_dep_helper
from concourse.bass_utils import run_bass_kernel_spmd

H = 112
W = 112
C = 21
N = H * W
NCORES = 8
YPC = H // NCORES            # 14 image rows per core
COLS = N // NCORES           # 1568 pixels per core
NB = 98                      # contraction blocks of 128 pixels
CTS = [(0, 512), (512, 512), (1024, 512), (1536, 32)]   # col tiles of 1568
NITER = 5
THETA_ALPHA = 160.0
THETA_BETA = 3.0
THETA_GAMMA = 3.0

F32 = mybir.dt.float32
F32R = mybir.dt.float32r
BF16 = mybir.dt.bfloat16
FP8 = mybir.dt.float8e4
EXPF = mybir.ActivationFunctionType.Exp

_CACHE = {}


def _build_program():
    nc = bacc.Bacc("TRN2", target_bir_lowering=False, debug=False, num_devices=NCORES)

    # Chain every PE matmul in emission order (ordering-only deps) so the
    # scheduler keeps same-weights matmuls adjacent for LDWEIGHTS dedup.
    _mm_state = {"prev": None}

    def mm(*args, **kwargs):
        inst = nc.tensor.matmul(*args, **kwargs)
        if _mm_state["prev"] is not None:
            _add_dep_helper(inst.ins, _mm_state["prev"].ins, sync=False,
                            reason="pe emission order")
        _mm_state["prev"] = inst
        return inst

    ub7 = nc.dram_tensor("ub7", [7, N], F32R, kind="ExternalInput")
    vb7 = nc.dram_tensor("vb7", [7, COLS], F32R, kind="ExternalInput")
    gxn = nc.dram_tensor("gxn", [W, W], BF16, kind="ExternalInput")
    gy2n = nc.dram_tensor("gy2n", [H, YPC], BF16, kind="ExternalInput")
    u21 = nc.dram_tensor("u21", [C, COLS], F32, kind="ExternalInput")
    qblk0 = nc.dram_tensor("qblk0", [128, NB * C], BF16, kind="ExternalInput")
    qy0 = nc.dram_tensor("qy0", [H, W * C], BF16, kind="ExternalInput")
    awT = nc.dram_tensor("awT", [54, C], F32, kind="ExternalInput")
    qt_out = nc.dram_tensor("qt_out", [C, COLS], F32, kind="ExternalOutput")

    with tile.TileContext(nc) as tc:
        with (
            tc.tile_pool(name="const", bufs=1) as cpool,
            tc.tile_pool(name="iter", bufs=1) as ipool,
            tc.tile_pool(name="stream", bufs=1) as stpool,
            tc.tile_pool(name="psum", bufs=1, space="PSUM") as pspool,
            tc.tile_pool(name="dram", bufs=1, space="DRAM") as dpool,
        ):
            # ---------------- constants ----------------
            gxn_sb = cpool.tile([W, W], BF16, tag="gxn", name="gxn")
            nc.sync.dma_start(gxn_sb[:], gxn[:])
            gy2n_sb = cpool.tile([H, YPC], BF16, tag="gy2n", name="gy2n")
            nc.sync.dma_start(gy2n_sb[:], gy2n[:])
            u21_sb = cpool.tile([C, COLS], F32, tag="u21", name="u21")
            nc.sync.dma_start(u21_sb[:], u21[:])
            awT_sb = cpool.tile([54, C], F32, tag="awT", name="awT")
            nc.sync.dma_start(awT_sb[:], awT[:])
            ones21 = cpool.tile([1, C], F32, tag="ones21", name="ones21")
            nc.gpsimd.memset(ones21[:], 1.0)
            invnb = cpool.tile([C, COLS], BF16, tag="invnb", name="invnb")

            # Q staged for the DVE 32x32 transpose: rows 21:32 stay zero.
            q32 = cpool.tile([32, COLS], BF16, tag="q32", name="q32")
            nc.gpsimd.memset(q32[:], 0.0)

            E_res = cpool.tile([128, NB, COLS], FP8, tag="eres", name="eres")

            qt_full = None

            def head(it):
                """Load Q, softmax block-major (bilateral lhsT) + y-major
                (spatial input). Returns (smB, smy)."""
                qblk = ipool.tile([128, NB * C], BF16, tag="qblk_smy",
                                  name=f"qblk_{it}")
                if it == 0:
                    nc.sync.dma_start(qblk[:], qblk0[:])
                else:
                    nc.sync.dma_start(
                        qblk[:].rearrange("r (b c) -> r b c", b=NB),
                        qt_full[:].rearrange("(r b) c -> r b c", r=128),
                    )
                eqB = ipool.tile([128, NB * C], BF16, tag="eqB_so54",
                                 name=f"eqB_{it}")
                nc.scalar.activation(eqB[:], qblk[:], EXPF)
                sums = ipool.tile([128, NB], F32, tag="sums", name=f"sums_{it}")
                nc.vector.reduce_sum(
                    sums[:], eqB[:].rearrange("p (b c) -> p b c", b=NB),
                    axis=mybir.AxisListType.X,
                )
                rsum = ipool.tile([128, NB], F32, tag="rsum", name=f"rsum_{it}")
                nc.vector.reciprocal(rsum[:], sums[:])
                smB = ipool.tile([128, NB, 33], BF16, tag="smB", name=f"smB_{it}")
                nc.gpsimd.memset(smB[:, :, C: 33], 1.0)
                nc.vector.tensor_mul(
                    smB[:, :, 0:C],
                    eqB[:].rearrange("p (b c) -> p b c", b=NB),
                    rsum[:].broadcast_to([128, NB, C]),
                )

                smy = ipool.tile([H, W * C], BF16, tag="qblk_smy",
                                 name=f"smy_{it}")
                if it == 0:
                    qy = ipool.tile([H, W * C], BF16, tag="qy_spc",
                                    name=f"qy_{it}")
                    nc.sync.dma_start(qy[:], qy0[:])
                    eqy = ipool.tile([H, W * C], BF16, tag="eqy_spT",
                                     name=f"eqy_{it}")
                    nc.scalar.activation(eqy[:], qy[:], EXPF)
                    sums2 = ipool.tile([H, W], F32, tag="sums", name=f"sums2_{it}")
                    nc.vector.reduce_sum(
                        sums2[:], eqy[:].rearrange("p (x c) -> p x c", x=W),
                        axis=mybir.AxisListType.X,
                    )
                    rsum2 = ipool.tile([H, W], F32, tag="rsum", name=f"rsum2_{it}")
                    nc.vector.reciprocal(rsum2[:], sums2[:])
                    nc.vector.tensor_mul(
                        smy[:].rearrange("p (x c) -> p x c", x=W),
                        eqy[:].rearrange("p (x c) -> p x c", x=W),
                        rsum2[:].broadcast_to([H, W, C]),
                    )
                else:
                    # compact copy -> both DMA sides fully contiguous
                    smc = ipool.tile([128, NB * C], BF16, tag="eqB_so54",
                                     name=f"smc_{it}")
                    nc.vector.tensor_copy(
                        smc[:].rearrange("p (b c) -> p b c", b=NB),
                        smB[:, :, 0:C],
                    )
                    sm_pm = dpool.tile([N, C], BF16, tag="smpm", bufs=2,
                                       name=f"smpm_{it}")
                    nc.sync.dma_start(
                        sm_pm[:].rearrange("(r b) (c) -> r (b c)", r=128), smc[:]
                    )
                    nc.sync.dma_start(
                        smy[:], sm_pm[:].rearrange("(y f) c -> y (f c)", y=H)
                    )
                return smB, smy

            # it0 head first: its ACT exp beats the 98 setup exps into the
            # strict-FIFO scalar queue, so iteration 0 starts right after
            # the d2 matmuls finish.
            head0 = head(0)

            # ---------------- setup: E_b = exp(d2) in fp8 ----------------
            vb7_sb = ipool.tile([7, COLS], F32R, tag="qT_qfin", name="vb7")
            nc.sync.dma_start(vb7_sb[:], vb7[:])
            for b in range(NB):
                if b % 2 == 0:
                    ubc = stpool.tile([7, 256], F32R, tag="ubc", bufs=2,
                                      name=f"ubc_{b}")
                    nc.sync.dma_start(
                        ubc[:, 0:min(256, (NB - b) * 128)],
                        ub7[:, b * 128: min(N, (b + 2) * 128)],
                    )
                ps = pspool.tile([128, 2048], F32, tag=("psA" if b % 2 == 0 else "psB"),
                                 name=f"d2_{b}")
                for ci, (c0, cw) in enumerate(CTS):
                    mm(ps[:, ci * 512: ci * 512 + cw],
                       ubc[:, (b % 2) * 128: (b % 2 + 1) * 128],
                       vb7_sb[:, c0: c0 + cw], start=True, stop=True)
                nc.scalar.activation(E_res[:, b, :], ps[:, 0:COLS], EXPF)

            # ---------------- iterations ----------------
            for it in range(NITER):
                smB, smy = head0 if it == 0 else head(it)

                psB = pspool.tile([128, 2048], F32, tag="psB", name=f"psB_{it}")
                bl_ps = pspool.tile([33, 2048], F32, tag="psA", name=f"bl_{it}")

                def bl_chunk(b0, b1):
                    for b in range(b0, b1):
                        for ci, (c0, cw) in enumerate(CTS):
                            mm(bl_ps[:, ci * 512: ci * 512 + cw],
                               smB[:, b, :], E_res[:, b, c0: c0 + cw],
                               start=(b == 0), stop=(b == NB - 1))

                bl_chunk(0, 56)

                # y-pass: Tk[k, (x c)] = gy2n^T @ smy  (y'-normalizer folded)
                Tk = ipool.tile([YPC, W * C], BF16, tag="Tk", name=f"Tk_{it}")
                XCH = [(0, 24), (24, 24), (48, 24), (72, 24), (96, 16)]
                for k, (x0, xw) in enumerate(XCH):
                    mm(psB[0:YPC, (k % 2) * 512: (k % 2) * 512 + xw * C],
                       gy2n_sb[:], smy[:, x0 * C: (x0 + xw) * C],
                       start=True, stop=True)
                    nc.vector.tensor_copy(
                        Tk[:, x0 * C: (x0 + xw) * C],
                        psB[0:YPC, (k % 2) * 512: (k % 2) * 512 + xw * C],
                    )
                # bounce to x-on-partitions (33K elems only)
                td = dpool.tile([YPC, W * C], BF16, tag="td", bufs=2, name=f"td_{it}")
                nc.sync.dma_start(td[:], Tk[:])
                Txk = ipool.tile([W, YPC * C], BF16, tag="Txk", name=f"Txk_{it}")
                nc.sync.dma_start(
                    Txk[:].rearrange("x (k c) -> x k c", k=YPC),
                    td[:].rearrange("k (x c) -> x k c", x=W),
                )

                bl_chunk(56, 80)

                # x-pass in 4 chunks of 28 output columns; each [28, (k c)]
                # PSUM block is padded to [32, (k, 32)] and DVE-32-transposed
                # into class partitions, then copied into the stacked rhs.
                so54 = ipool.tile([54, COLS], F32, tag="eqB_so54",
                                  name=f"so54_{it}")
                for j, x0 in enumerate([0, 28, 56, 84]):
                    pslab = psB[0:28, 1024 + (j % 2) * 512:
                                 1024 + (j % 2) * 512 + YPC * C]
                    mm(pslab, gxn_sb[:, x0: x0 + 28], Txk[:],
                       start=True, stop=True)
                    spc = ipool.tile([32, YPC * 32], BF16, tag="qy_spc",
                                     name=f"spc_{it}_{j}")
                    nc.gpsimd.memset(spc[:], 0.0)
                    nc.vector.tensor_copy(
                        spc[:].rearrange("p (k c) -> p k c", c=32)[0:28, :, 0:C],
                        pslab.rearrange("p (k c) -> p k c", c=C),
                    )
                    spT = ipool.tile([32, YPC * 32], BF16, tag="eqy_spT",
                                     name=f"spT_{it}_{j}")
                    nc.vector.transpose(spT[:], spc[:])
                    nc.vector.tensor_copy(
                        so54[32: 32 + C, :]
                        .rearrange("c (k x) -> c k x", k=YPC)[:, :, x0: x0 + 28],
                        spT[:].rearrange("p (k x) -> p k x", x=32)[0:C, :, 0:28],
                    )

                bl_chunk(80, NB)

                # ---- iteration 0: invnb = 1/nb broadcast across class rows
                if it == 0:
                    rnb = ipool.tile([1, COLS], F32, tag="qT_qfin", name="rnb")
                    nc.vector.reciprocal(rnb[:], bl_ps[32:33, 0:COLS])
                    for ci, (c0, cw) in enumerate(CTS):
                        mm(psB[0:C, ci * 512: ci * 512 + cw],
                           ones21[:], rnb[:, c0: c0 + cw], start=True, stop=True)
                    nc.vector.tensor_copy(invnb[:], psB[0:C, 0:COLS])

                # ---- stacked rhs rows 0:21 bilateral (rows 21:32 zero)
                nc.gpsimd.memset(so54[0:32, :], 0.0)
                nc.vector.tensor_mul(so54[0:C, :], bl_ps[0:C, 0:COLS], invnb[:])

                # ---- Q = u + [A_b ; A_s]^T @ so54
                q_ps = pspool.tile([C, 2048], F32, tag="psA", name=f"qps_{it}")
                for ci, (c0, cw) in enumerate(CTS):
                    mm(q_ps[:, ci * 512: ci * 512 + cw],
                       awT_sb[:], so54[:, c0: c0 + cw], start=True, stop=True)

                if it < NITER - 1:
                    nc.vector.tensor_add(q32[0:C, :], q_ps[:, 0:COLS], u21_sb[:])
                    qT = ipool.tile([32, COLS], BF16, tag="qT_qfin", name=f"qT_{it}")
                    nc.vector.transpose(qT[:], q32[:])
                    qt_sl = dpool.tile([COLS, C], BF16, tag="qtsl", bufs=2,
                                       name=f"qtsl_{it}")
                    nc.sync.dma_start(
                        qt_sl[:].rearrange("(k p) c -> p k c", p=32),
                        qT[:].rearrange("p (k c) -> p k c", c=32)[:, :, 0:C],
                    )
                    qt_full = dpool.tile([N, C], BF16, tag="qtfull", bufs=2,
                                         addr_space="Shared", name=f"qtfull_{it}")
                    nc.gpsimd.collective_compute(
                        "AllGather",
                        mybir.AluOpType.bypass,
                        replica_groups=[list(range(NCORES))],
                        ins=[qt_sl[:]],
                        outs=[qt_full[:]],
                    )
                else:
                    q_fin = ipool.tile([C, COLS], F32, tag="qT_qfin", name="qfin")
                    nc.vector.tensor_add(q_fin[:], q_ps[:, 0:COLS], u21_sb[:])
                    nc.sync.dma_start(qt_out[:], q_fin[:])

    nc.compile()
    return nc


def _host_inputs(unaries, rgb, spatial_kernel, bilateral_kernel, compatibility_matrix):
    bf = ml_dtypes.bfloat16
    u = np.transpose(np.asarray(unaries, dtype=np.float32)[0], (2, 0, 1)).reshape(C, N)
    rgbf = np.asarray(rgb, dtype=np.float32)[0].reshape(N, 3)

    yy, xx = np.meshgrid(
        np.arange(H, dtype=np.float64), np.arange(W, dtype=np.float64), indexing="ij"
    )
    pos = np.stack([xx.ravel(), yy.ravel()], axis=1)  # [N, 2] (x, y)

    fb = np.concatenate(
        [pos / THETA_ALPHA, rgbf.astype(np.float64) / THETA_BETA], axis=1
    )
    fb -= fb.mean(axis=0, keepdims=True)  # centering: reduces fp32 cancellation
    fb32 = fb.astype(np.float32)
    sq = (fb32.astype(np.float64) ** 2).sum(axis=1)
    mhalf_sq = (-0.5 * sq).astype(np.float32)

    ub7_np = np.empty((7, N), np.float32)
    ub7_np[0:5] = fb32.T
    ub7_np[5] = mhalf_sq
    ub7_np[6] = 1.0
    vb7_np = np.empty((7, N), np.float32)
    vb7_np[0:5] = fb32.T
    vb7_np[5] = 1.0
    vb7_np[6] = mhalf_sq

    d = np.arange(-(H - 1), H, dtype=np.float64)
    g1tab = np.exp(-(d * d) / (2.0 * THETA_GAMMA**2))

    def g1(dd):
        return g1tab[np.asarray(dd) + (H - 1)]

    G = g1(np.arange(W)[:, None] - np.arange(W)[None, :])  # [t, t']
    s1 = np.array([g1(np.arange(H) - t).sum() for t in range(H)])
    gxn_np = np.ascontiguousarray((G / s1[None, :]).astype(bf))  # [x, x']

    comp = np.asarray(compatibility_matrix, dtype=np.float64)
    A_s = -(comp @ np.asarray(spatial_kernel, dtype=np.float64))
    A_b = -(comp @ np.asarray(bilateral_kernel, dtype=np.float64))
    awT_np = np.zeros((54, C), np.float32)
    awT_np[0:C] = A_b.T.astype(np.float32)
    awT_np[32: 32 + C] = A_s.T.astype(np.float32)

    uT = np.ascontiguousarray(u.T)  # [N, C]
    # contraction block b holds pixels {p*98+b}; DMA column (b,p) = pixel 98p+b
    X = (98 * np.arange(128)[None, :] + np.arange(NB)[:, None]).reshape(-1)
    ub7_np = np.ascontiguousarray(ub7_np[:, X])
    qblk0_np = np.ascontiguousarray(
        uT.reshape(128, NB, C).reshape(128, NB * C).astype(bf)
    )
    qy0_np = np.ascontiguousarray(uT.reshape(H, W * C).astype(bf))

    in_maps = []
    for c in range(NCORES):
        sl = slice(c * COLS, (c + 1) * COLS)
        dy = np.arange(H)[:, None] - (YPC * c + np.arange(YPC))[None, :]  # [y, k]
        gy2n_np = np.ascontiguousarray(
            (g1(dy) / s1[YPC * c + np.arange(YPC)][None, :]).astype(bf)
        )
        in_maps.append(
            dict(
                ub7=ub7_np,
                vb7=np.ascontiguousarray(vb7_np[:, sl]),
                gxn=gxn_np,
                gy2n=gy2n_np,
                u21=np.ascontiguousarray(u[:, sl]),
                qblk0=qblk0_np,
                qy0=qy0_np,
                awT=awT_np,
            )
        )
    return in_maps


def run(inputs, trace=False, **spmd_kwargs):
    in_maps = _host_inputs(**inputs)
    if "nc" not in _CACHE:
        _CACHE["nc"] = _build_program()
    nc = _CACHE["nc"]
    res = run_bass_kernel_spmd(
        nc, in_maps, core_ids=list(range(NCORES)), trace=trace, **spmd_kwargs
    )
    qs = [np.asarray(res.results[c]["qt_out"]) for c in range(NCORES)]
    Q = np.concatenate(qs, axis=1)  # [C, N]
    out = Q.reshape(C, H, W).transpose(1, 2, 0)[None].astype(np.float32)
    return out, res


def kernel(unaries, rgb, spatial_kernel, bilateral_kernel, compatibility_matrix):
    out, _ = run(
        dict(
            unaries=unaries,
            rgb=rgb,
            spatial_kernel=spatial_kernel,
            bilateral_kernel=bilateral_kernel,
            compatibility_matrix=compatibility_matrix,
        )
    )
    return out
